# revision 14
# baseline (speedup 1.0000x reference)
"""Distributed Trainium2 Bass kernel for nn_Block_32332513804635 (moe_routing).

Transformer block: LN -> 8-head attention (alibi+causal) -> residual -> LN ->
MoE (16 routed experts, top-6, SwiGLU) + shared expert -> residual.

Sharding over 8 NeuronCores (SPMD, one graph; per-core differences via data):
  - LN1/LN2/gating/attention out-projection: replicated over full tokens
    (streamed per 128-token tile) -- trades idle-engine compute for the
    removal of three collective barriers.
  - attention: head-parallel (1 head/core), AllGather of per-head outputs.
  - routed experts: expert-parallel, 2 experts/core (cap 640 "big" + cap 384
    "small", pairing balances measured loads); on-device top-6 routing,
    dispatch via dma_gather(transpose), combine via gating-scaled
    dma_scatter_add into bf16 partial buffers split in two D-halves.
  - shared expert: intermediate-dim-parallel (2816 padded to 3072 = 8*384).
  - final: two ReduceScatters (one per D-half, first overlaps expert
    compute), residual selected via host-fed one-hot accumulate.

kernel(**inputs) takes FULL unsharded inputs, returns the FULL output.
"""
import numpy as np
import ml_dtypes

import concourse.bacc as bacc
import concourse.tile as tile
import concourse.mybir as mybir
import concourse.library_config as library_config
from concourse.bass_utils import run_bass_kernel_spmd

BF = mybir.dt.bfloat16
FP = mybir.dt.float32
I16 = mybir.dt.int16
AF = mybir.ActivationFunctionType
ALU = mybir.AluOpType
AX = mybir.AxisListType

bf16 = ml_dtypes.bfloat16

NCORES = 8
T, D = 1024, 2048
HD = D // 2                          # D-half for the split combine
H, DK, DV = 8, 128, 128
E, TOPK, F = 16, 6, 1408
FS, FS_PAD = 2816, 3072
FS_SLICE = FS_PAD // NCORES          # 384
NFT_S = FS_SLICE // 128              # 3
NFT = F // 128                       # 11
NDT = D // 128                       # 16
NTT = T // 128                       # 8
TS = T // NCORES                     # 128
CAP_A, CAP_B = 640, 384
CAP = CAP_A + CAP_B
NIT_A, NIT_B = CAP_A // 128, CAP_B // 128
EPS = 1e-8
A_EXPERTS = [3, 5, 13, 0, 4, 9, 12, 14]
B_EXPERTS = [10, 11, 15, 1, 2, 6, 7, 8]

_CACHE = {}

_INPUT_SPECS = [
    ("x_full", [T, D], FP), ("am", [T, T], FP),
    ("wq", [D, DK], BF), ("wk", [D, DK], BF), ("wv", [D, DV], BF),
    ("bq", [1, DK], BF), ("bk", [1, DK], BF), ("bv", [1, DV], BF),
    ("wo", [H * DV, D], BF), ("wo_b", [1, D], BF),
    ("g1", [128, D], FP), ("g2", [128, D], FP),
    ("gate_wT", [D, E], FP), ("gate_b3", [128, NTT, E], FP),
    ("selA", [128, E], FP), ("selB", [128, E], FP),
    ("slice_sel", [128, NTT], FP),
    ("w1A", [NFT, 128, NDT, 128], BF), ("w3A", [NFT, 128, NDT, 128], BF),
    ("w2A", [NFT, 128, D], BF),
    ("b1A", [1, NFT, 128], BF), ("b3A", [1, NFT, 128], BF), ("b2A", [1, D], BF),
    ("w1B", [NFT, 128, NDT, 128], BF), ("w3B", [NFT, 128, NDT, 128], BF),
    ("w2B", [NFT, 128, D], BF),
    ("b1B", [1, NFT, 128], BF), ("b3B", [1, NFT, 128], BF), ("b2B", [1, D], BF),
    ("ws1", [NFT_S, 128, NDT, 128], BF), ("ws3", [NFT_S, 128, NDT, 128], BF),
    ("ws2", [NFT_S, 128, D], BF),
    ("bs1", [1, NFT_S, 128], BF), ("bs3", [1, NFT_S, 128], BF),
    ("bs2_8", [1, D], BF),
    ("ident", [128, 128], BF), ("ident_f", [128, 128], FP),
    ("tri_incl", [128, 128], FP), ("tri_s8", [8, 8], FP),
    ("ones8", [8, 128], FP), ("ones_col", [128, 1], FP),
    ("iota_bc", [128, CAP_A], FP), ("iota_t", [128, NTT], FP),
    ("ones_row", [1, 1024], BF),
]


def _build_nc():
    nc = bacc.Bacc("TRN2", target_bir_lowering=False, debug=False,
                   num_devices=NCORES, num_swdge_queues=2)
    t = {}
    for name, shape, dt in _INPUT_SPECS:
        t[name] = nc.dram_tensor(name, list(shape), dt, kind="ExternalInput")
    out_ext = nc.dram_tensor("out", [TS, D], FP, kind="ExternalOutput")

    d_oT = nc.dram_tensor("d_oT", [DV, T], BF)
    ag_oT = nc.dram_tensor("ag_oT", [H * DV, T], BF, addr_space="Shared")
    d_x3n = nc.dram_tensor("d_x3n", [T, D], BF)
    d_idx = nc.dram_tensor("d_idx", [CAP], I16)
    d_yp0 = nc.dram_tensor("d_yp0", [T, HD], BF)
    d_yp1 = nc.dram_tensor("d_yp1", [T, HD], BF)
    d_rs0 = nc.dram_tensor("d_rs0", [TS, HD], BF)
    d_rs1 = nc.dram_tensor("d_rs1", [TS, HD], BF)

    rg = [list(range(NCORES))]

    with tile.TileContext(nc) as tc:
        with tc.tile_pool(name="cpool", bufs=1) as cp, \
             tc.tile_pool(name="ppool", bufs=1) as pp:

            nc.gpsimd.load_library(library_config.mlp)

            def load(pool, name):
                src = t[name]
                tl = pool.tile(list(src.shape), src.dtype, tag=name, name=name)
                nc.sync.dma_start(out=tl[:], in_=src[:])
                return tl

            ident = load(cp, "ident")
            ident_f = load(cp, "ident_f")
            tri_incl = load(cp, "tri_incl")
            tri_s8 = load(cp, "tri_s8")
            ones8 = load(cp, "ones8")
            ones_col = load(cp, "ones_col")
            iota_bc = load(cp, "iota_bc")
            iota_t = load(cp, "iota_t")
            ones_row = load(cp, "ones_row")
            g1 = load(cp, "g1")
            g2 = load(cp, "g2")
            gate_b3 = load(cp, "gate_b3")
            selA = load(cp, "selA")
            selB = load(cp, "selB")
            slice_sel = load(cp, "slice_sel")
            gate_w_sb = cp.tile([128, NDT, E], FP, tag="gate_w")
            nc.sync.dma_start(out=gate_w_sb[:], in_=t["gate_wT"][:].rearrange(
                "(dt p) e -> p dt e", p=128))

            x2_mine = pp.tile([128, D], FP, tag="x2_mine")
            nc.vector.memset(x2_mine[:], 0.0)
            wd_sb = pp.tile([128, NTT, E], FP, tag="wd")
            x3T = pp.tile([128, NDT, T], BF, tag="x3T")

            def layer_norm(pool, src, gb, dst):
                s = pool.tile([128, 1], FP, tag="ln_s", name="ln_s")
                nc.vector.tensor_reduce(s[:], src[:], AX.X, ALU.add)
                negmu = pool.tile([128, 1], FP, tag="ln_negmu", name="ln_negmu")
                nc.vector.tensor_scalar_mul(negmu[:], s[:], -1.0 / D)
                sq = pool.tile([128, D], FP, tag="ln_tmp", bufs=1, name="ln_sq")
                ssq = pool.tile([128, 1], FP, tag="ln_ssq", name="ln_ssq")
                nc.scalar.activation(sq[:], src[:], AF.Square,
                                     bias=negmu[:], accum_out=ssq[:])
                var = pool.tile([128, 1], FP, tag="ln_var", name="ln_var")
                nc.vector.tensor_scalar(var[:], ssq[:], 1.0 / D, EPS,
                                        ALU.mult, ALU.add)
                sd = pool.tile([128, 1], FP, tag="ln_sd", name="ln_sd")
                nc.scalar.activation(sd[:], var[:], AF.Sqrt)
                rstd = pool.tile([128, 1], FP, tag="ln_rstd", name="ln_rstd")
                nc.vector.reciprocal(rstd[:], sd[:])
                tmp = pool.tile([128, D], FP, tag="ln_tmp", bufs=1, name="ln_tmp")
                nc.vector.scalar_tensor_tensor(tmp[:], src[:], negmu[:],
                                               gb[:], ALU.add, ALU.mult)
                nc.vector.tensor_scalar_mul(dst[:], tmp[:], rstd[:])

            # =============================================================
            # Phase 1+2: LN1 (all tokens, streamed) + attention head
            # =============================================================
            with tc.tile_pool(name="apool", bufs=1) as ap, \
                 tc.tile_pool(name="amp", bufs=2) as amp, \
                 tc.tile_pool(name="pst", bufs=2, space="PSUM") as pst:

                x1T = ap.tile([128, NDT, T], BF, tag="x1T")
                for tt in range(NTT):
                    x_t = amp.tile([128, D], FP, tag="x_t", name="x_t")
                    nc.sync.dma_start(out=x_t[:],
                                      in_=t["x_full"][tt * 128:(tt + 1) * 128, :])
                    x1 = amp.tile([128, D], FP, tag="x1", name="x1")
                    layer_norm(amp, x_t, g1, x1)
                    x1b = amp.tile([128, D], BF, tag="x1b", name="x1b")
                    nc.vector.tensor_copy(out=x1b[:], in_=x1[:])
                    for dt in range(NDT):
                        pt = pst.tile([128, 128], BF, tag="ps_tr", name="pt")
                        nc.tensor.transpose(
                            pt[:], x1b[:, dt * 128:(dt + 1) * 128], ident[:])
                        nc.vector.tensor_copy(
                            out=x1T[:, dt, tt * 128:(tt + 1) * 128], in_=pt[:])

                # ---- attention head ----
                cm2 = tc.tile_pool(name="ps2", bufs=2, space="PSUM")
                ps2 = cm2.__enter__()
                wq_sb = ap.tile([128, NDT, DK], BF, tag="wq")
                nc.sync.dma_start(out=wq_sb[:], in_=t["wq"][:].rearrange(
                    "(dt p) f -> p dt f", p=128))
                wk_sb = ap.tile([128, NDT, DK], BF, tag="wk")
                nc.sync.dma_start(out=wk_sb[:], in_=t["wk"][:].rearrange(
                    "(dt p) f -> p dt f", p=128))
                wv_sb = ap.tile([128, NDT, DV], BF, tag="wv")
                nc.sync.dma_start(out=wv_sb[:], in_=t["wv"][:].rearrange(
                    "(dt p) f -> p dt f", p=128))
                bq = load(ap, "bq")
                bk = load(ap, "bk")
                bv = load(ap, "bv")

                qT = ap.tile([128, T], BF, tag="qT")
                kT = ap.tile([128, T], BF, tag="kT")
                for dst, w_sb, b_sb in ((qT, wq_sb, bq), (kT, wk_sb, bk)):
                    for c in range(2):
                        sl = slice(c * 512, (c + 1) * 512)
                        ps = ps2.tile([128, 512], FP, tag="ps_qk", name="ps_qk")
                        for dt in range(NDT):
                            nc.tensor.matmul(ps[:], w_sb[:, dt, :],
                                             x1T[:, dt, sl],
                                             start=(dt == 0), stop=False)
                        nc.tensor.matmul(ps[:], b_sb[:], ones_row[:, :512],
                                         start=False, stop=True)
                        nc.scalar.activation(dst[:, sl], ps[:], AF.Copy)

                v_sb = ap.tile([128, NTT, DV], BF, tag="v_sb")
                for tt in range(NTT):
                    ps = ps2.tile([128, DV], FP, tag="ps_v", name="ps_v")
                    for dt in range(NDT):
                        nc.tensor.matmul(
                            ps[:], x1T[:, dt, tt * 128:(tt + 1) * 128],
                            wv_sb[:, dt, :], start=(dt == 0), stop=False)
                    nc.tensor.matmul(ps[:], ones_row[:, :128], bv[:],
                                     start=False, stop=True)
                    nc.scalar.activation(v_sb[:, tt, :], ps[:], AF.Copy)

                cm2.__exit__(None, None, None)
                cm3 = tc.tile_pool(name="ps3", bufs=2, space="PSUM")
                ps3 = cm3.__enter__()
                p_sb = ap.tile([128, NTT, T], BF, tag="p_sb")
                for tt in range(NTT):
                    am_t = amp.tile([128, T], FP, tag="am_t", name="am_t")
                    nc.sync.dma_start(out=am_t[:],
                                      in_=t["am"][tt * 128:(tt + 1) * 128, :])
                    s_sb = amp.tile([128, T], FP, tag="s_sb", name="s_sb")
                    for c in range(2):
                        sl = slice(c * 512, (c + 1) * 512)
                        ps = ps3.tile([128, 512], FP, tag="ps_s", name="ps_s")
                        nc.tensor.matmul(ps[:], qT[:, tt * 128:(tt + 1) * 128],
                                         kT[:, sl], start=True, stop=True)
                        nc.vector.scalar_tensor_tensor(
                            s_sb[:, sl], ps[:], DK ** -0.5, am_t[:, sl],
                            ALU.mult, ALU.add)
                    negmax = amp.tile([128, 1], FP, tag="negmax", name="negmax")
                    nc.vector.tensor_reduce(negmax[:], s_sb[:], AX.X, ALU.max,
                                            negate=True)
                    sumexp = amp.tile([128, 1], FP, tag="sumexp", name="sumexp")
                    nc.scalar.activation(p_sb[:, tt, :], s_sb[:], AF.Exp,
                                         bias=negmax[:], accum_out=sumexp[:])
                    rec = amp.tile([128, 1], FP, tag="rec", name="rec")
                    nc.vector.reciprocal(rec[:], sumexp[:])
                    nc.vector.tensor_scalar_mul(v_sb[:, tt, :], v_sb[:, tt, :],
                                                rec[:])

                oT = ap.tile([128, T], BF, tag="oT")
                for c in range(2):
                    sl = slice(c * 512, (c + 1) * 512)
                    ps = ps3.tile([128, 512], FP, tag="ps_o", name="ps_o")
                    for tt in range(NTT):
                        nc.tensor.matmul(ps[:], v_sb[:, tt, :], p_sb[:, tt, sl],
                                         start=(tt == 0), stop=(tt == NTT - 1))
                    nc.scalar.activation(oT[:, sl], ps[:], AF.Copy)
                nc.sync.dma_start(out=d_oT[:], in_=oT[:])
                nc.gpsimd.collective_compute(
                    "AllGather", ALU.bypass, replica_groups=rg,
                    ins=[d_oT[:]], outs=[ag_oT[:]])
                cm3.__exit__(None, None, None)

            # =============================================================
            # Phase 3: out-projection + x2 + LN2 + gating (all tokens)
            # =============================================================
            with tc.tile_pool(name="bpool", bufs=1) as bp, \
                 tc.tile_pool(name="bmp", bufs=2) as bmp, \
                 tc.tile_pool(name="ps4", bufs=1, space="PSUM") as ps4:

                oT_all = bp.tile([128, H, T], BF, tag="oT_all")
                nc.sync.dma_start(out=oT_all[:], in_=ag_oT[:].rearrange(
                    "(ht p) s -> p ht s", p=128))
                wo_sb = bp.tile([128, H, D], BF, tag="wo_sb")
                nc.sync.dma_start(out=wo_sb[:], in_=t["wo"][:].rearrange(
                    "(ht p) d -> p ht d", p=128))
                wo_b = load(bp, "wo_b")
                logits = bp.tile([128, NTT, E], FP, tag="logits")

                for tt in range(NTT):
                    tsl = slice(tt * 128, (tt + 1) * 128)
                    x2_t = bmp.tile([128, D], FP, tag="x2_t", name="x2_t")
                    x_t2 = bmp.tile([128, D], FP, tag="x_t2", bufs=1, name="x_t2")
                    nc.sync.dma_start(out=x_t2[:], in_=t["x_full"][tsl, :])
                    for dc in range(4):
                        sl = slice(dc * 512, (dc + 1) * 512)
                        ps = ps4.tile([128, 512], FP, tag=f"ps_x2{dc % 2}",
                                      bufs=2, name="ps_x2")
                        for ht in range(H):
                            nc.tensor.matmul(ps[:], oT_all[:, ht, tsl],
                                             wo_sb[:, ht, sl],
                                             start=(ht == 0), stop=False)
                        nc.tensor.matmul(ps[:], ones_row[:, :128],
                                         wo_b[:, sl], start=False, stop=True)
                        nc.vector.tensor_tensor(out=x2_t[:, sl], in0=ps[:],
                                                in1=x_t2[:, sl], op=ALU.add)
                    nc.vector.scalar_tensor_tensor(
                        x2_mine[:], x2_t[:], slice_sel[:, tt:tt + 1],
                        x2_mine[:], ALU.mult, ALU.add)
                    x3 = bmp.tile([128, D], FP, tag="x3", name="x3")
                    layer_norm(bmp, x2_t, g2, x3)
                    x3b = bmp.tile([128, D], BF, tag="x3b", bufs=1, name="x3b")
                    nc.vector.tensor_copy(out=x3b[:], in_=x3[:])
                    nc.sync.dma_start(out=d_x3n[tsl, :], in_=x3b[:])
                    x3Tf = bmp.tile([128, NDT, 128], FP, tag="x3Tf", bufs=1,
                                    name="x3Tf")
                    for dt in range(NDT):
                        ptf = ps4.tile([128, 128], FP, tag="ps_tr", bufs=2,
                                       name="ptf")
                        nc.tensor.transpose(
                            ptf[:], x3[:, dt * 128:(dt + 1) * 128], ident_f[:])
                        nc.vector.tensor_copy(out=x3Tf[:, dt, :], in_=ptf[:])
                    nc.vector.tensor_copy(out=x3T[:, :, tsl], in_=x3Tf[:])
                    ps_lg = ps4.tile([128, E], FP, tag="ps_lgt", bufs=2,
                                     name="ps_lg")
                    for dt in range(NDT):
                        nc.tensor.matmul(ps_lg[:], x3Tf[:, dt, :],
                                         gate_w_sb[:, dt, :],
                                         start=(dt == 0), stop=(dt == NDT - 1))
                    nc.vector.tensor_copy(out=logits[:, tt, :], in_=ps_lg[:])

                # batched softmax + top-6 over [128, NTT, E]
                mx = bp.tile([128, NTT], FP, tag="g_mx")
                nc.vector.tensor_reduce(mx[:], logits[:], AX.X, ALU.max)
                sh = bp.tile([128, NTT, E], FP, tag="g_sh")
                nc.vector.tensor_tensor(out=sh[:], in0=logits[:],
                                        in1=mx[:].broadcast_to([128, NTT, E]),
                                        op=ALU.subtract)
                ex = bp.tile([128, NTT, E], FP, tag="g_ex")
                nc.scalar.activation(ex[:], sh[:], AF.Exp)
                se = bp.tile([128, NTT], FP, tag="g_se")
                nc.vector.tensor_reduce(se[:], ex[:], AX.X, ALU.add)
                rec = bp.tile([128, NTT], FP, tag="g_rec")
                nc.vector.reciprocal(rec[:], se[:])
                sm = bp.tile([128, NTT, E], FP, tag="g_sm")
                nc.vector.tensor_tensor(out=sm[:], in0=ex[:],
                                        in1=rec[:].broadcast_to([128, NTT, E]),
                                        op=ALU.mult)
                cur = bp.tile([128, NTT, E], FP, tag="g_cur")
                nc.vector.tensor_tensor(out=cur[:], in0=sm[:], in1=gate_b3[:],
                                        op=ALU.add)
                nc.vector.memset(wd_sb[:], 0.0)
                for _ in range(TOPK):
                    mx2 = bp.tile([128, NTT], FP, tag="g_mx2", name="g_mx2")
                    nc.vector.tensor_reduce(mx2[:], cur[:], AX.X, ALU.max)
                    oh = bp.tile([128, NTT, E], FP, tag="g_oh", name="g_oh")
                    nc.vector.tensor_tensor(
                        out=oh[:], in0=cur[:],
                        in1=mx2[:].broadcast_to([128, NTT, E]), op=ALU.is_ge)
                    t1 = bp.tile([128, NTT, E], FP, tag="g_t1", name="g_t1")
                    nc.vector.tensor_tensor(out=t1[:], in0=oh[:], in1=sm[:],
                                            op=ALU.mult)
                    nc.vector.tensor_tensor(out=wd_sb[:], in0=wd_sb[:],
                                            in1=t1[:], op=ALU.add)
                    nc.vector.scalar_tensor_tensor(cur[:], oh[:], -1e30,
                                                   cur[:], ALU.mult, ALU.add)

            # =============================================================
            # Phase 4: routing, gather, shared expert, routed experts
            # =============================================================
            with tc.tile_pool(name="mpool", bufs=1) as mp, \
                 tc.tile_pool(name="wsp", bufs=2) as wsp, \
                 tc.tile_pool(name="psm", bufs=1, space="PSUM") as psm:

                # ---- routing ----
                cmr = tc.tile_pool(name="psr", bufs=1, space="PSUM")
                psr = cmr.__enter__()
                gw_its = []
                for sfx, sel_oh, cap, nit, base in (
                        ("A", selA, CAP_A, NIT_A, 0),
                        ("B", selB, CAP_B, NIT_B, CAP_A)):
                    wdcol = mp.tile([128, NTT], FP, tag=f"wdcol{sfx}",
                                    name=f"wdcol{sfx}")
                    for tt in range(NTT):
                        tsel = mp.tile([128, E], FP, tag="r_tsel", name="r_tsel")
                        nc.vector.tensor_tensor(out=tsel[:], in0=wd_sb[:, tt, :],
                                                in1=sel_oh[:], op=ALU.mult)
                        nc.vector.tensor_reduce(wdcol[:, tt:tt + 1], tsel[:],
                                                AX.X, ALU.add)
                    mask = mp.tile([128, NTT], FP, tag=f"mask{sfx}",
                                   name=f"mask{sfx}")
                    nc.vector.tensor_scalar(mask[:], wdcol[:], 0.0, None,
                                            ALU.is_gt)
                    ps_tot = psr.tile([8, 1], FP, tag="ps_ri", name="ps_tot")
                    nc.tensor.matmul(ps_tot[:], mask[:], ones_col[:],
                                     start=True, stop=True)
                    tot = mp.tile([8, 1], FP, tag="r_tot", name="r_tot")
                    nc.vector.tensor_copy(out=tot[:], in_=ps_tot[:])
                    rhs8 = mp.tile([8, 8], FP, tag="r_rhs8", name="r_rhs8")
                    nc.vector.tensor_scalar_mul(rhs8[:], tri_s8[:], tot[:])
                    ps_cum = psr.tile([128, NTT], FP, tag="ps_ri", name="ps_cum")
                    nc.tensor.matmul(ps_cum[:], tri_incl[:], mask[:],
                                     start=True, stop=False)
                    nc.tensor.matmul(ps_cum[:], ones8[:], rhs8[:],
                                     start=False, stop=True)
                    pos = mp.tile([128, NTT], FP, tag="r_pos", name="r_pos")
                    nc.scalar.activation(pos[:], ps_cum[:], AF.Copy, bias=-1.0)
                    posm = mp.tile([128, NTT], FP, tag="r_posm", name="r_posm")
                    nc.vector.scalar_tensor_tensor(posm[:], pos[:], 5.0,
                                                   mask[:], ALU.add, ALU.mult)
                    nc.vector.tensor_scalar_add(posm[:], posm[:], -5.0)

                    chunks = [(0, 512), (512, cap)] if cap > 512 else [(0, cap)]
                    ps_l2s = [psr.tile([2, hi - lo], FP, tag="ps_l2", bufs=2,
                                       name=f"ps_l2_{sfx}{ci}")
                              for ci, (lo, hi) in enumerate(chunks)]
                    for tt in range(NTT):
                        g_t = mp.tile([128, cap], FP, tag=f"r_g{sfx}",
                                      name=f"r_g{sfx}")
                        nc.vector.tensor_scalar(g_t[:], iota_bc[:, :cap],
                                                posm[:, tt:tt + 1], None,
                                                ALU.is_equal)
                        rhs2 = mp.tile([128, 2], FP, tag="r_rhs2", name="r_rhs2")
                        nc.vector.tensor_copy(out=rhs2[:, 0:1],
                                              in_=iota_t[:, tt:tt + 1])
                        nc.vector.tensor_copy(out=rhs2[:, 1:2],
                                              in_=wdcol[:, tt:tt + 1])
                        for ci, (lo, hi) in enumerate(chunks):
                            nc.tensor.matmul(ps_l2s[ci][:], rhs2[:],
                                             g_t[:, lo:hi],
                                             start=(tt == 0),
                                             stop=(tt == NTT - 1))
                    lg2 = mp.tile([2, cap], FP, tag=f"r_lg2{sfx}",
                                  name=f"r_lg2{sfx}")
                    for ci, (lo, hi) in enumerate(chunks):
                        nc.vector.tensor_copy(out=lg2[:, lo:hi],
                                              in_=ps_l2s[ci][:])
                    for it in range(nit):
                        pslt = psr.tile([128, 2], FP, tag="ps_lgT", name="pslt")
                        nc.tensor.transpose(pslt[:],
                                            lg2[:, it * 128:(it + 1) * 128],
                                            ident_f[:2, :2])
                        lgit = mp.tile([128, 2], FP, tag=f"r_lgit{sfx}{it}",
                                       name=f"r_lgit{sfx}{it}")
                        nc.vector.tensor_copy(out=lgit[:], in_=pslt[:])
                        gw_its.append(lgit)
                        i16 = mp.tile([128, 1], I16, tag="r_i16", name="r_i16")
                        nc.vector.tensor_copy(out=i16[:], in_=lgit[:, 0:1])
                        off = base + it * 128
                        nc.sync.dma_start(out=d_idx[off:off + 128], in_=i16[:])

                idx_sb = mp.tile([128, CAP // 16], I16, tag="idx_sb")
                for r in range(8):
                    nc.sync.dma_start(
                        out=idx_sb[16 * r:16 * (r + 1), :],
                        in_=d_idx[:].rearrange("(c q) -> q c", q=16))
                cmr.__exit__(None, None, None)

                XeT_A = mp.tile([128, NDT, CAP_A], BF, tag="XeT_A")
                nc.gpsimd.dma_gather(
                    out_ap=XeT_A[:], in_ap=d_x3n[:],
                    idxs_ap=idx_sb[:, :CAP_A // 16],
                    num_idxs=CAP_A, num_idxs_reg=CAP_A, elem_size=D,
                    transpose=True, queue_num=0)
                XeT_B = mp.tile([128, NDT, CAP_B], BF, tag="XeT_B")
                nc.gpsimd.dma_gather(
                    out_ap=XeT_B[:], in_ap=d_x3n[:],
                    idxs_ap=idx_sb[:, CAP_A // 16:],
                    num_idxs=CAP_B, num_idxs_reg=CAP_B, elem_size=D,
                    transpose=True, queue_num=1)
                XeTs = {"A": XeT_A, "B": XeT_B}

                # ---- shared expert (initializes d_yp0/d_yp1) ----
                hs = mp.tile([128, NFT_S, T], BF, tag="hs")
                for ft in range(NFT_S):
                    ws1_t = wsp.tile([128, NDT, 128], BF, tag="w1t",
                                     name="ws1_t")
                    nc.sync.dma_start(out=ws1_t[:], in_=t["ws1"][ft])
                    ws3_t = wsp.tile([128, NDT, 128], BF, tag="w3t",
                                     name="ws3_t")
                    nc.sync.dma_start(out=ws3_t[:], in_=t["ws3"][ft])
                    for c in range(2):
                        sl = slice(c * 512, (c + 1) * 512)
                        ph1 = psm.tile([128, 512], FP, tag="ps_h1", name="ph1")
                        ph3 = psm.tile([128, 512], FP, tag="ps_h3", name="ph3")
                        for dt in range(NDT):
                            nc.tensor.matmul(ph1[:], ws1_t[:, dt, :],
                                             x3T[:, dt, sl],
                                             start=(dt == 0), stop=False)
                            nc.tensor.matmul(ph3[:], ws3_t[:, dt, :],
                                             x3T[:, dt, sl],
                                             start=(dt == 0), stop=False)
                        b_s1 = wsp.tile([1, 128], BF, tag="b1t", name="b_s1")
                        nc.sync.dma_start(out=b_s1[:], in_=t["bs1"][0:1, ft, :])
                        b_s3 = wsp.tile([1, 128], BF, tag="b3t", name="b_s3")
                        nc.sync.dma_start(out=b_s3[:], in_=t["bs3"][0:1, ft, :])
                        nc.tensor.matmul(ph1[:], b_s1[:], ones_row[:, :512],
                                         start=False, stop=True)
                        nc.tensor.matmul(ph3[:], b_s3[:], ones_row[:, :512],
                                         start=False, stop=True)
                        sg = mp.tile([128, 512], BF, tag="sg", name="sg")
                        nc.scalar.activation(sg[:], ph1[:], AF.Sigmoid)
                        a_t = mp.tile([128, 512], BF, tag="a_t", name="a_t")
                        nc.vector.scalar_tensor_tensor(a_t[:], ph1[:], 1.0,
                                                       sg[:], ALU.mult,
                                                       ALU.mult)
                        nc.vector.tensor_tensor(out=hs[:, ft, sl], in0=a_t[:],
                                                in1=ph3[:], op=ALU.mult)
                ws2_sb = mp.tile([128, NFT_S, D], BF, tag="ws2_sb")
                nc.sync.dma_start(out=ws2_sb[:],
                                  in_=t["ws2"][:].rearrange("f p d -> p f d"))
                for tt in range(NTT):
                    ys = wsp.tile([128, D], BF, tag="ys", name="ys")
                    for dc in range(4):
                        sl = slice(dc * 512, (dc + 1) * 512)
                        pys = psm.tile([128, 512], FP, tag="ps_ys", name="pys")
                        for ft in range(NFT_S):
                            nc.tensor.matmul(
                                pys[:], hs[:, ft, tt * 128:(tt + 1) * 128],
                                ws2_sb[:, ft, sl],
                                start=(ft == 0), stop=False)
                        b_s2 = wsp.tile([1, 512], BF, tag="b2t", name="b_s2")
                        nc.sync.dma_start(out=b_s2[:], in_=t["bs2_8"][0:1, sl])
                        nc.tensor.matmul(pys[:], ones_row[:, :128],
                                         b_s2[:], start=False, stop=True)
                        nc.scalar.activation(ys[:, sl], pys[:], AF.Copy)
                    tsl = slice(tt * 128, (tt + 1) * 128)
                    nc.sync.dma_start(out=d_yp0[tsl, :], in_=ys[:, :HD])
                    nc.sync.dma_start(out=d_yp1[tsl, :], in_=ys[:, HD:])

                # ---- routed experts: h for both, then ye per D-half ----
                hTs = {}
                for sfx, cap in (("A", CAP_A), ("B", CAP_B)):
                    XeT = XeTs[sfx]
                    hT = mp.tile([128, NFT, cap], BF, tag=f"hT{sfx}",
                                 name=f"hT{sfx}")
                    hTs[sfx] = hT
                    chunks = [(0, 512), (512, cap)] if cap > 512 else [(0, cap)]
                    for ft in range(NFT):
                        w1_t = wsp.tile([128, NDT, 128], BF, tag="w1t",
                                        name="w1_t")
                        nc.sync.dma_start(out=w1_t[:], in_=t[f"w1{sfx}"][ft])
                        w3_t = wsp.tile([128, NDT, 128], BF, tag="w3t",
                                        name="w3_t")
                        nc.sync.dma_start(out=w3_t[:], in_=t[f"w3{sfx}"][ft])
                        for (lo, hi) in chunks:
                            w = hi - lo
                            ph1 = psm.tile([128, 512], FP, tag="ps_h1",
                                           name="ph1")
                            ph3 = psm.tile([128, 512], FP, tag="ps_h3",
                                           name="ph3")
                            for dt in range(NDT):
                                nc.tensor.matmul(
                                    ph1[:, :w], w1_t[:, dt, :],
                                    XeT[:, dt, lo:hi],
                                    start=(dt == 0), stop=False)
                                nc.tensor.matmul(
                                    ph3[:, :w], w3_t[:, dt, :],
                                    XeT[:, dt, lo:hi],
                                    start=(dt == 0), stop=False)
                            b_1 = wsp.tile([1, 128], BF, tag="b1t", name="b_1")
                            nc.sync.dma_start(out=b_1[:],
                                              in_=t[f"b1{sfx}"][0:1, ft, :])
                            b_3 = wsp.tile([1, 128], BF, tag="b3t", name="b_3")
                            nc.sync.dma_start(out=b_3[:],
                                              in_=t[f"b3{sfx}"][0:1, ft, :])
                            nc.tensor.matmul(ph1[:, :w], b_1[:],
                                             ones_row[:, :w],
                                             start=False, stop=True)
                            nc.tensor.matmul(ph3[:, :w], b_3[:],
                                             ones_row[:, :w],
                                             start=False, stop=True)
                            sg = mp.tile([128, 512], BF, tag="sg", name="sg")
                            nc.scalar.activation(sg[:, :w], ph1[:, :w],
                                                 AF.Sigmoid)
                            a_t = mp.tile([128, 512], BF, tag="a_t", name="a_t")
                            nc.vector.scalar_tensor_tensor(
                                a_t[:, :w], ph1[:, :w], 1.0, sg[:, :w],
                                ALU.mult, ALU.mult)
                            nc.vector.tensor_tensor(
                                out=hT[:, ft, lo:hi], in0=a_t[:, :w],
                                in1=ph3[:, :w], op=ALU.mult)

                cmy = tc.tile_pool(name="psy", bufs=1, space="PSUM")
                psy = cmy.__enter__()
                for half, d_yph, dcs in ((0, d_yp0, (0, 1)), (1, d_yp1, (2, 3))):
                    for sfx, cap, nit, base, it_base in (
                            ("A", CAP_A, NIT_A, 0, 0),
                            ("B", CAP_B, NIT_B, CAP_A, NIT_A)):
                        hT = hTs[sfx]
                        ye = mp.tile([128, nit, HD], BF, tag="ye",
                                     name=f"ye{sfx}{half}")
                        for dci, dc in enumerate(dcs):
                            sl = slice(dc * 512, (dc + 1) * 512)
                            osl = slice(dci * 512, (dci + 1) * 512)
                            pyes = [psy.tile([128, 512], FP, tag=f"ps_ye{i}",
                                             name=f"ps_ye_{sfx}{dc}_{i}")
                                    for i in range(nit)]
                            for ft in range(NFT):
                                w2_t = wsp.tile([128, 512], BF, tag="w2t",
                                                name="w2_t")
                                nc.sync.dma_start(out=w2_t[:],
                                                  in_=t[f"w2{sfx}"][ft, :, sl])
                                for it in range(nit):
                                    nc.tensor.matmul(
                                        pyes[it][:],
                                        hT[:, ft, it * 128:(it + 1) * 128],
                                        w2_t[:], start=(ft == 0), stop=False)
                            b_2 = wsp.tile([1, 512], BF, tag="b2t", name="b_2")
                            nc.sync.dma_start(out=b_2[:],
                                              in_=t[f"b2{sfx}"][0:1, sl])
                            for it in range(nit):
                                nc.tensor.matmul(pyes[it][:], ones_row[:, :128],
                                                 b_2[:], start=False, stop=True)
                                nc.scalar.activation(
                                    ye[:, it, osl], pyes[it][:], AF.Copy,
                                    scale=gw_its[it_base + it][:, 1:2])
                        nc.gpsimd.dma_scatter_add(
                            out_ap=d_yph[:], in_ap=ye[:],
                            idxs_ap=idx_sb[:, base // 16:(base + cap) // 16],
                            num_idxs=cap, num_idxs_reg=cap, elem_size=HD,
                            queue_num=(0 if sfx == "A" else 1))
                    d_rsh = d_rs0 if half == 0 else d_rs1
                    nc.gpsimd.collective_compute(
                        "ReduceScatter", ALU.add, replica_groups=rg,
                        ins=[(d_yp0 if half == 0 else d_yp1)[:]],
                        outs=[d_rsh[:]])
                cmy.__exit__(None, None, None)

                # ---- final: residual + output ----
                rs_sb = mp.tile([128, D], BF, tag="rs_sb")
                nc.sync.dma_start(out=rs_sb[:, :HD], in_=d_rs0[:])
                nc.sync.dma_start(out=rs_sb[:, HD:], in_=d_rs1[:])
                nc.vector.tensor_tensor(out=x2_mine[:], in0=rs_sb[:],
                                        in1=x2_mine[:], op=ALU.add)
                nc.sync.dma_start(out=out_ext[:], in_=x2_mine[:])

    nc.compile()
    return nc


# --------------------------------------------------------------------------
# host-side input prep
# --------------------------------------------------------------------------

def _tile_w1(w):
    nft = w.shape[1] // 128
    return np.ascontiguousarray(
        w.reshape(NDT, 128, nft, 128).transpose(2, 1, 0, 3))


def _prep_in_maps(inputs):
    f32 = lambda a: np.ascontiguousarray(np.asarray(a, dtype=np.float32))
    tobf = lambda a: np.ascontiguousarray(np.asarray(a, dtype=np.float32)
                                          .astype(bf16))
    x = f32(inputs["x"]).reshape(T, D)
    mask = f32(inputs["mask"])
    wq_w, wq_b = f32(inputs["wq_w"]), f32(inputs["wq_b"])
    wk_w, wk_b = f32(inputs["wk_w"]), f32(inputs["wk_b"])
    wv_w, wv_b = f32(inputs["wv_w"]), f32(inputs["wv_b"])
    wo_w, wo_b = f32(inputs["wo_w"]), f32(inputs["wo_b"])
    attn_g, ffn_g = f32(inputs["attn_g"]), f32(inputs["ffn_g"])
    gate_w, gate_b = f32(inputs["gate_w"]), f32(inputs["gate_b"])
    e_w1, e_b1 = f32(inputs["e_w1"]), f32(inputs["e_b1"])
    e_w2, e_b2 = f32(inputs["e_w2"]), f32(inputs["e_b2"])
    e_w3, e_b3 = f32(inputs["e_w3"]), f32(inputs["e_b3"])
    s_w1, s_b1 = f32(inputs["s_w1"]), f32(inputs["s_b1"])
    s_w2, s_b2 = f32(inputs["s_w2"]), f32(inputs["s_b2"])
    s_w3, s_b3 = f32(inputs["s_w3"]), f32(inputs["s_b3"])

    s_w1p = np.zeros((D, FS_PAD), np.float32); s_w1p[:, :FS] = s_w1
    s_w3p = np.zeros((D, FS_PAD), np.float32); s_w3p[:, :FS] = s_w3
    s_b1p = np.zeros(FS_PAD, np.float32); s_b1p[:FS] = s_b1
    s_b3p = np.zeros(FS_PAD, np.float32); s_b3p[:FS] = s_b3
    s_w2p = np.zeros((FS_PAD, D), np.float32); s_w2p[:FS] = s_w2

    i_idx = np.arange(T)[:, None]
    j_idx = np.arange(T)[None, :]
    rel = np.where(i_idx >= j_idx, -(i_idx - j_idx).astype(np.float32), 0.0)
    ident = np.eye(128, dtype=np.float32)
    tri_incl = (np.arange(128)[:, None] <= np.arange(128)[None, :]) \
        .astype(np.float32)
    tri_s8 = (np.arange(8)[:, None] < np.arange(8)[None, :]).astype(np.float32)
    iota_bc = np.tile(np.arange(CAP_A, dtype=np.float32), (128, 1))
    iota_t = (np.arange(NTT)[None, :] * 128
              + np.arange(128)[:, None]).astype(np.float32)

    in_maps = []
    for c in range(NCORES):
        eA, eB = A_EXPERTS[c], B_EXPERTS[c]
        slope = 2.0 ** (-(c + 1))
        selA = np.zeros(E, np.float32); selA[eA] = 1.0
        selB = np.zeros(E, np.float32); selB[eB] = 1.0
        ssel = np.zeros(NTT, np.float32); ssel[c] = 1.0
        fs_lo = c * FS_SLICE
        fs_hi = fs_lo + FS_SLICE
        m = {
            "x_full": x,
            "am": (mask + slope * rel).astype(np.float32),
            "wq": tobf(wq_w[:, c * DK:(c + 1) * DK]),
            "wk": tobf(wk_w[:, c * DK:(c + 1) * DK]),
            "wv": tobf(wv_w[:, c * DV:(c + 1) * DV]),
            "bq": tobf(wq_b[c * DK:(c + 1) * DK]).reshape(1, DK),
            "bk": tobf(wk_b[c * DK:(c + 1) * DK]).reshape(1, DK),
            "bv": tobf(wv_b[c * DV:(c + 1) * DV]).reshape(1, DV),
            "wo": tobf(wo_w),
            "wo_b": tobf(wo_b).reshape(1, D),
            "g1": np.tile(attn_g, (128, 1)),
            "g2": np.tile(ffn_g, (128, 1)),
            "gate_wT": np.ascontiguousarray(gate_w.T),
            "gate_b3": np.tile(gate_b, (128, NTT, 1)).astype(np.float32),
            "selA": np.tile(selA, (128, 1)),
            "selB": np.tile(selB, (128, 1)),
            "slice_sel": np.tile(ssel, (128, 1)),
            "ws1": _tile_w1(tobf(s_w1p[:, fs_lo:fs_hi])),
            "ws3": _tile_w1(tobf(s_w3p[:, fs_lo:fs_hi])),
            "ws2": tobf(s_w2p[fs_lo:fs_hi]).reshape(NFT_S, 128, D),
            "bs1": tobf(s_b1p[fs_lo:fs_hi]).reshape(1, NFT_S, 128),
            "bs3": tobf(s_b3p[fs_lo:fs_hi]).reshape(1, NFT_S, 128),
            "bs2_8": tobf(s_b2 / 8.0).reshape(1, D),
            "ident": ident.astype(bf16),
            "ident_f": ident,
            "tri_incl": tri_incl,
            "tri_s8": tri_s8,
            "ones8": np.ones((8, 128), np.float32),
            "ones_col": np.ones((128, 1), np.float32),
            "iota_bc": iota_bc,
            "iota_t": iota_t,
            "ones_row": np.ones((1, 1024), bf16),
        }
        for sfx, e in (("A", eA), ("B", eB)):
            m[f"w1{sfx}"] = _tile_w1(tobf(e_w1[e]))
            m[f"w3{sfx}"] = _tile_w1(tobf(e_w3[e]))
            m[f"w2{sfx}"] = tobf(e_w2[e]).reshape(NFT, 128, D)
            m[f"b1{sfx}"] = tobf(e_b1[e]).reshape(1, NFT, 128)
            m[f"b3{sfx}"] = tobf(e_b3[e]).reshape(1, NFT, 128)
            m[f"b2{sfx}"] = tobf(e_b2[e]).reshape(1, D)
        in_maps.append(m)
    return in_maps


def _get_nc():
    if "nc" not in _CACHE:
        _CACHE["nc"] = _build_nc()
    return _CACHE["nc"]


def kernel(trace=False, **inputs):
    nc = _get_nc()
    in_maps = _prep_in_maps(inputs)
    res = run_bass_kernel_spmd(nc, in_maps, core_ids=list(range(NCORES)),
                               trace=trace)
    out = np.concatenate([res.results[c]["out"] for c in range(NCORES)],
                         axis=0).reshape(1, T, D).astype(np.float32)
    if trace:
        return out, res
    return out


# revision 16
# speedup vs baseline: 1.0586x; 1.0586x over previous
"""Distributed Trainium2 Bass kernel for nn_Block_32332513804635 (moe_routing).

Transformer block: LN -> 8-head attention (alibi+causal) -> residual -> LN ->
MoE (16 routed experts, top-6, SwiGLU) + shared expert -> residual.

Sharding over 8 NeuronCores (SPMD, one graph; per-core differences via data):
  - LN1/LN2/gating/attention out-projection: replicated over full tokens
    (streamed per 128-token tile) -- trades idle-engine compute for the
    removal of three collective barriers.
  - attention: head-parallel (1 head/core), AllGather of per-head outputs.
  - routed experts: expert-parallel, 2 experts/core (cap 640 "big" + cap 384
    "small", pairing balances measured loads); on-device top-6 routing,
    dispatch via dma_gather(transpose), combine via gating-scaled
    dma_scatter_add into bf16 partial buffers split in two D-halves.
  - shared expert: intermediate-dim-parallel (2816 padded to 3072 = 8*384).
  - final: two ReduceScatters (one per D-half, first overlaps expert
    compute), residual selected via host-fed one-hot accumulate.

kernel(**inputs) takes FULL unsharded inputs, returns the FULL output.
"""
import numpy as np
import ml_dtypes

import concourse.bacc as bacc
import concourse.tile as tile
import concourse.mybir as mybir
import concourse.library_config as library_config
from concourse.bass_utils import run_bass_kernel_spmd

BF = mybir.dt.bfloat16
FP = mybir.dt.float32
I16 = mybir.dt.int16
F8 = mybir.dt.float8e4
PM = mybir.MatmulPerfMode
AF = mybir.ActivationFunctionType
ALU = mybir.AluOpType
AX = mybir.AxisListType

bf16 = ml_dtypes.bfloat16
fp8 = mybir.dt.np(mybir.dt.float8e4)
WS = 1024.0            # fp8 weight scale
NFTP = 6               # ft pairs for DoubleRow ye (11 tiles + 1 zero pad)

NCORES = 8
T, D = 1024, 2048
HD = D // 2                          # D-half for the split combine
H, DK, DV = 8, 128, 128
E, TOPK, F = 16, 6, 1408
FS, FS_PAD = 2816, 3072
FS_SLICE = FS_PAD // NCORES          # 384
NFT_S = FS_SLICE // 128              # 3
NFT = F // 128                       # 11
NDT = D // 128                       # 16
NTT = T // 128                       # 8
TS = T // NCORES                     # 128
CAP_A, CAP_B = 640, 384
CAP = CAP_A + CAP_B
NIT_A, NIT_B = CAP_A // 128, CAP_B // 128
EPS = 1e-8
A_EXPERTS = [3, 5, 13, 0, 4, 9, 12, 14]
B_EXPERTS = [10, 11, 15, 1, 2, 6, 7, 8]

_CACHE = {}

_INPUT_SPECS = [
    ("x_full", [T, D], FP), ("am", [T, T], FP),
    ("wq", [D, DK], BF), ("wk", [D, DK], BF), ("wv", [D, DV], BF),
    ("bq", [1, DK], BF), ("bk", [1, DK], BF), ("bv", [1, DV], BF),
    ("wo", [H * DV, D], BF), ("wo_b", [1, D], BF),
    ("g1", [128, D], FP), ("g2", [128, D], FP),
    ("gate_wT", [D, E], FP), ("gate_b3", [128, NTT, E], FP),
    ("selA", [128, E], FP), ("selB", [128, E], FP),
    ("slice_sel", [128, NTT], FP),
    ("w1A", [NFT, 128, 8, 2, 128], F8), ("w3A", [NFT, 128, 8, 2, 128], F8),
    ("w2A", [NFTP, 128, 2, D], F8),
    ("b1A", [1, NFT, 128], BF), ("b3A", [1, NFT, 128], BF), ("b2A", [1, D], BF),
    ("w1B", [NFT, 128, 8, 2, 128], F8), ("w3B", [NFT, 128, 8, 2, 128], F8),
    ("w2B", [NFTP, 128, 2, D], F8),
    ("b1B", [1, NFT, 128], BF), ("b3B", [1, NFT, 128], BF), ("b2B", [1, D], BF),
    ("ws1", [NFT_S, 128, NDT, 128], BF), ("ws3", [NFT_S, 128, NDT, 128], BF),
    ("ws2", [NFT_S, 128, D], BF),
    ("bs1", [1, NFT_S, 128], BF), ("bs3", [1, NFT_S, 128], BF),
    ("bs2_8", [1, D], BF),
    ("ident", [128, 128], BF), ("ident_f", [128, 128], FP),
    ("tri_incl", [128, 128], FP), ("tri_s8", [8, 8], FP),
    ("ones8", [8, 128], FP), ("ones_col", [128, 1], FP),
    ("iota_bc", [128, CAP_A], FP), ("iota_t", [128, NTT], FP),
    ("ones_row", [1, 1024], BF),
]


def _build_nc():
    nc = bacc.Bacc("TRN2", target_bir_lowering=False, debug=False,
                   num_devices=NCORES, num_swdge_queues=2)
    t = {}
    for name, shape, dt in _INPUT_SPECS:
        t[name] = nc.dram_tensor(name, list(shape), dt, kind="ExternalInput")
    out_ext = nc.dram_tensor("out", [TS, D], FP, kind="ExternalOutput")

    d_oT = nc.dram_tensor("d_oT", [DV, T], BF)
    ag_oT = nc.dram_tensor("ag_oT", [H * DV, T], BF, addr_space="Shared")
    d_x3n = nc.dram_tensor("d_x3n", [T, D], F8)
    d_idx = nc.dram_tensor("d_idx", [CAP], I16)
    d_yp0 = nc.dram_tensor("d_yp0", [T, HD], BF)
    d_yp1 = nc.dram_tensor("d_yp1", [T, HD], BF)
    d_rs0 = nc.dram_tensor("d_rs0", [TS, HD], BF)
    d_rs1 = nc.dram_tensor("d_rs1", [TS, HD], BF)

    rg = [list(range(NCORES))]

    with tile.TileContext(nc) as tc:
        with tc.tile_pool(name="cpool", bufs=1) as cp, \
             tc.tile_pool(name="ppool", bufs=1) as pp:

            nc.gpsimd.load_library(library_config.mlp)

            def load(pool, name):
                src = t[name]
                tl = pool.tile(list(src.shape), src.dtype, tag=name, name=name)
                nc.sync.dma_start(out=tl[:], in_=src[:])
                return tl

            ident = load(cp, "ident")
            ident_f = load(cp, "ident_f")
            tri_incl = load(cp, "tri_incl")
            tri_s8 = load(cp, "tri_s8")
            ones8 = load(cp, "ones8")
            ones_col = load(cp, "ones_col")
            iota_bc = load(cp, "iota_bc")
            iota_t = load(cp, "iota_t")
            ones_row = load(cp, "ones_row")
            g1 = load(cp, "g1")
            g2 = load(cp, "g2")
            gate_b3 = load(cp, "gate_b3")
            selA = load(cp, "selA")
            selB = load(cp, "selB")
            slice_sel = load(cp, "slice_sel")
            gate_w_sb = cp.tile([128, NDT, E], FP, tag="gate_w")
            nc.sync.dma_start(out=gate_w_sb[:], in_=t["gate_wT"][:].rearrange(
                "(dt p) e -> p dt e", p=128))

            x2_mine = pp.tile([128, D], FP, tag="x2_mine")
            nc.vector.memset(x2_mine[:], 0.0)
            wd_sb = pp.tile([128, NTT, E], FP, tag="wd")
            x3T = pp.tile([128, NDT, T], BF, tag="x3T")

            def layer_norm(pool, src, gb, dst):
                s = pool.tile([128, 1], FP, tag="ln_s", name="ln_s")
                nc.vector.tensor_reduce(s[:], src[:], AX.X, ALU.add)
                negmu = pool.tile([128, 1], FP, tag="ln_negmu", name="ln_negmu")
                nc.vector.tensor_scalar_mul(negmu[:], s[:], -1.0 / D)
                sq = pool.tile([128, D], FP, tag="ln_tmp", bufs=1, name="ln_sq")
                ssq = pool.tile([128, 1], FP, tag="ln_ssq", name="ln_ssq")
                nc.scalar.activation(sq[:], src[:], AF.Square,
                                     bias=negmu[:], accum_out=ssq[:])
                var = pool.tile([128, 1], FP, tag="ln_var", name="ln_var")
                nc.vector.tensor_scalar(var[:], ssq[:], 1.0 / D, EPS,
                                        ALU.mult, ALU.add)
                sd = pool.tile([128, 1], FP, tag="ln_sd", name="ln_sd")
                nc.scalar.activation(sd[:], var[:], AF.Sqrt)
                rstd = pool.tile([128, 1], FP, tag="ln_rstd", name="ln_rstd")
                nc.vector.reciprocal(rstd[:], sd[:])
                tmp = pool.tile([128, D], FP, tag="ln_tmp", bufs=1, name="ln_tmp")
                nc.vector.scalar_tensor_tensor(tmp[:], src[:], negmu[:],
                                               gb[:], ALU.add, ALU.mult)
                nc.vector.tensor_scalar_mul(dst[:], tmp[:], rstd[:])

            # =============================================================
            # Phase 1+2: LN1 (all tokens, streamed) + attention head
            # =============================================================
            with tc.tile_pool(name="apool", bufs=1) as ap, \
                 tc.tile_pool(name="amp", bufs=2) as amp, \
                 tc.tile_pool(name="pst", bufs=2, space="PSUM") as pst:

                x1T = ap.tile([128, NDT, T], BF, tag="x1T")
                for tt in range(NTT):
                    x_t = amp.tile([128, D], FP, tag="x_t", name="x_t")
                    nc.sync.dma_start(out=x_t[:],
                                      in_=t["x_full"][tt * 128:(tt + 1) * 128, :])
                    x1 = amp.tile([128, D], FP, tag="x1", name="x1")
                    layer_norm(amp, x_t, g1, x1)
                    x1b = amp.tile([128, D], BF, tag="x1b", name="x1b")
                    nc.vector.tensor_copy(out=x1b[:], in_=x1[:])
                    for dt in range(NDT):
                        pt = pst.tile([128, 128], BF, tag="ps_tr", name="pt")
                        nc.tensor.transpose(
                            pt[:], x1b[:, dt * 128:(dt + 1) * 128], ident[:])
                        nc.vector.tensor_copy(
                            out=x1T[:, dt, tt * 128:(tt + 1) * 128], in_=pt[:])

                # ---- attention head ----
                cm2 = tc.tile_pool(name="ps2", bufs=2, space="PSUM")
                ps2 = cm2.__enter__()
                wq_sb = ap.tile([128, NDT, DK], BF, tag="wq")
                nc.sync.dma_start(out=wq_sb[:], in_=t["wq"][:].rearrange(
                    "(dt p) f -> p dt f", p=128))
                wk_sb = ap.tile([128, NDT, DK], BF, tag="wk")
                nc.sync.dma_start(out=wk_sb[:], in_=t["wk"][:].rearrange(
                    "(dt p) f -> p dt f", p=128))
                wv_sb = ap.tile([128, NDT, DV], BF, tag="wv")
                nc.sync.dma_start(out=wv_sb[:], in_=t["wv"][:].rearrange(
                    "(dt p) f -> p dt f", p=128))
                bq = load(ap, "bq")
                bk = load(ap, "bk")
                bv = load(ap, "bv")

                qT = ap.tile([128, T], BF, tag="qT")
                kT = ap.tile([128, T], BF, tag="kT")
                for dst, w_sb, b_sb in ((qT, wq_sb, bq), (kT, wk_sb, bk)):
                    for c in range(2):
                        sl = slice(c * 512, (c + 1) * 512)
                        ps = ps2.tile([128, 512], FP, tag="ps_qk", name="ps_qk")
                        for dt in range(NDT):
                            nc.tensor.matmul(ps[:], w_sb[:, dt, :],
                                             x1T[:, dt, sl],
                                             start=(dt == 0), stop=False)
                        nc.tensor.matmul(ps[:], b_sb[:], ones_row[:, :512],
                                         start=False, stop=True)
                        nc.scalar.activation(dst[:, sl], ps[:], AF.Copy)

                v_sb = ap.tile([128, NTT, DV], BF, tag="v_sb")
                for tt in range(NTT):
                    ps = ps2.tile([128, DV], FP, tag="ps_v", name="ps_v")
                    for dt in range(NDT):
                        nc.tensor.matmul(
                            ps[:], x1T[:, dt, tt * 128:(tt + 1) * 128],
                            wv_sb[:, dt, :], start=(dt == 0), stop=False)
                    nc.tensor.matmul(ps[:], ones_row[:, :128], bv[:],
                                     start=False, stop=True)
                    nc.scalar.activation(v_sb[:, tt, :], ps[:], AF.Copy)

                cm2.__exit__(None, None, None)
                cm3 = tc.tile_pool(name="ps3", bufs=2, space="PSUM")
                ps3 = cm3.__enter__()
                p_sb = ap.tile([128, NTT, T], BF, tag="p_sb")
                for tt in range(NTT):
                    am_t = amp.tile([128, T], FP, tag="am_t", name="am_t")
                    nc.sync.dma_start(out=am_t[:],
                                      in_=t["am"][tt * 128:(tt + 1) * 128, :])
                    s_sb = amp.tile([128, T], FP, tag="s_sb", name="s_sb")
                    for c in range(2):
                        sl = slice(c * 512, (c + 1) * 512)
                        ps = ps3.tile([128, 512], FP, tag="ps_s", name="ps_s")
                        nc.tensor.matmul(ps[:], qT[:, tt * 128:(tt + 1) * 128],
                                         kT[:, sl], start=True, stop=True)
                        nc.vector.scalar_tensor_tensor(
                            s_sb[:, sl], ps[:], DK ** -0.5, am_t[:, sl],
                            ALU.mult, ALU.add)
                    negmax = amp.tile([128, 1], FP, tag="negmax", name="negmax")
                    nc.vector.tensor_reduce(negmax[:], s_sb[:], AX.X, ALU.max,
                                            negate=True)
                    sumexp = amp.tile([128, 1], FP, tag="sumexp", name="sumexp")
                    nc.scalar.activation(p_sb[:, tt, :], s_sb[:], AF.Exp,
                                         bias=negmax[:], accum_out=sumexp[:])
                    rec = amp.tile([128, 1], FP, tag="rec", name="rec")
                    nc.vector.reciprocal(rec[:], sumexp[:])
                    nc.vector.tensor_scalar_mul(v_sb[:, tt, :], v_sb[:, tt, :],
                                                rec[:])

                oT = ap.tile([128, T], BF, tag="oT")
                for c in range(2):
                    sl = slice(c * 512, (c + 1) * 512)
                    ps = ps3.tile([128, 512], FP, tag="ps_o", name="ps_o")
                    for tt in range(NTT):
                        nc.tensor.matmul(ps[:], v_sb[:, tt, :], p_sb[:, tt, sl],
                                         start=(tt == 0), stop=(tt == NTT - 1))
                    nc.scalar.activation(oT[:, sl], ps[:], AF.Copy)
                nc.sync.dma_start(out=d_oT[:], in_=oT[:])
                nc.gpsimd.collective_compute(
                    "AllGather", ALU.bypass, replica_groups=rg,
                    ins=[d_oT[:]], outs=[ag_oT[:]])
                cm3.__exit__(None, None, None)

            # =============================================================
            # Phase 3: out-projection + x2 + LN2 + gating (all tokens)
            # =============================================================
            with tc.tile_pool(name="bpool", bufs=1) as bp, \
                 tc.tile_pool(name="bmp", bufs=2) as bmp, \
                 tc.tile_pool(name="ps4", bufs=1, space="PSUM") as ps4:

                oT_all = bp.tile([128, H, T], BF, tag="oT_all")
                nc.sync.dma_start(out=oT_all[:], in_=ag_oT[:].rearrange(
                    "(ht p) s -> p ht s", p=128))
                wo_sb = bp.tile([128, H, D], BF, tag="wo_sb")
                nc.sync.dma_start(out=wo_sb[:], in_=t["wo"][:].rearrange(
                    "(ht p) d -> p ht d", p=128))
                wo_b = load(bp, "wo_b")
                logits = bp.tile([128, NTT, E], FP, tag="logits")

                for tt in range(NTT):
                    tsl = slice(tt * 128, (tt + 1) * 128)
                    x2_t = bmp.tile([128, D], FP, tag="x2_t", name="x2_t")
                    x_t2 = bmp.tile([128, D], FP, tag="x_t2", bufs=1, name="x_t2")
                    nc.sync.dma_start(out=x_t2[:], in_=t["x_full"][tsl, :])
                    for dc in range(4):
                        sl = slice(dc * 512, (dc + 1) * 512)
                        ps = ps4.tile([128, 512], FP, tag=f"ps_x2{dc % 2}",
                                      bufs=2, name="ps_x2")
                        for ht in range(H):
                            nc.tensor.matmul(ps[:], oT_all[:, ht, tsl],
                                             wo_sb[:, ht, sl],
                                             start=(ht == 0), stop=False)
                        nc.tensor.matmul(ps[:], ones_row[:, :128],
                                         wo_b[:, sl], start=False, stop=True)
                        nc.vector.tensor_tensor(out=x2_t[:, sl], in0=ps[:],
                                                in1=x_t2[:, sl], op=ALU.add)
                    nc.vector.scalar_tensor_tensor(
                        x2_mine[:], x2_t[:], slice_sel[:, tt:tt + 1],
                        x2_mine[:], ALU.mult, ALU.add)
                    x3 = bmp.tile([128, D], FP, tag="x3", name="x3")
                    layer_norm(bmp, x2_t, g2, x3)
                    x3b = bmp.tile([128, D], F8, tag="x3b", bufs=1, name="x3b")
                    nc.vector.tensor_copy(out=x3b[:], in_=x3[:])
                    nc.sync.dma_start(out=d_x3n[tsl, :], in_=x3b[:])
                    x3Tf = bmp.tile([128, NDT, 128], FP, tag="x3Tf", bufs=1,
                                    name="x3Tf")
                    for dt in range(NDT):
                        ptf = ps4.tile([128, 128], FP, tag="ps_tr", bufs=2,
                                       name="ptf")
                        nc.tensor.transpose(
                            ptf[:], x3[:, dt * 128:(dt + 1) * 128], ident_f[:])
                        nc.vector.tensor_copy(out=x3Tf[:, dt, :], in_=ptf[:])
                    nc.vector.tensor_copy(out=x3T[:, :, tsl], in_=x3Tf[:])
                    ps_lg = ps4.tile([128, E], FP, tag="ps_lgt", bufs=2,
                                     name="ps_lg")
                    for dt in range(NDT):
                        nc.tensor.matmul(ps_lg[:], x3Tf[:, dt, :],
                                         gate_w_sb[:, dt, :],
                                         start=(dt == 0), stop=(dt == NDT - 1))
                    nc.vector.tensor_copy(out=logits[:, tt, :], in_=ps_lg[:])

                # batched softmax + top-6 over [128, NTT, E]
                mx = bp.tile([128, NTT], FP, tag="g_mx")
                nc.vector.tensor_reduce(mx[:], logits[:], AX.X, ALU.max)
                sh = bp.tile([128, NTT, E], FP, tag="g_sh")
                nc.vector.tensor_tensor(out=sh[:], in0=logits[:],
                                        in1=mx[:].broadcast_to([128, NTT, E]),
                                        op=ALU.subtract)
                ex = bp.tile([128, NTT, E], FP, tag="g_ex")
                nc.scalar.activation(ex[:], sh[:], AF.Exp)
                se = bp.tile([128, NTT], FP, tag="g_se")
                nc.vector.tensor_reduce(se[:], ex[:], AX.X, ALU.add)
                rec = bp.tile([128, NTT], FP, tag="g_rec")
                nc.vector.reciprocal(rec[:], se[:])
                sm = bp.tile([128, NTT, E], FP, tag="g_sm")
                nc.vector.tensor_tensor(out=sm[:], in0=ex[:],
                                        in1=rec[:].broadcast_to([128, NTT, E]),
                                        op=ALU.mult)
                cur = bp.tile([128, NTT, E], FP, tag="g_cur")
                nc.vector.tensor_tensor(out=cur[:], in0=sm[:], in1=gate_b3[:],
                                        op=ALU.add)
                nc.vector.memset(wd_sb[:], 0.0)
                for _ in range(TOPK):
                    mx2 = bp.tile([128, NTT], FP, tag="g_mx2", name="g_mx2")
                    nc.vector.tensor_reduce(mx2[:], cur[:], AX.X, ALU.max)
                    oh = bp.tile([128, NTT, E], FP, tag="g_oh", name="g_oh")
                    nc.vector.tensor_tensor(
                        out=oh[:], in0=cur[:],
                        in1=mx2[:].broadcast_to([128, NTT, E]), op=ALU.is_ge)
                    t1 = bp.tile([128, NTT, E], FP, tag="g_t1", name="g_t1")
                    nc.vector.tensor_tensor(out=t1[:], in0=oh[:], in1=sm[:],
                                            op=ALU.mult)
                    nc.vector.tensor_tensor(out=wd_sb[:], in0=wd_sb[:],
                                            in1=t1[:], op=ALU.add)
                    nc.vector.scalar_tensor_tensor(cur[:], oh[:], -1e30,
                                                   cur[:], ALU.mult, ALU.add)

            # =============================================================
            # Phase 4: routing, gather, shared expert, routed experts
            # =============================================================
            with tc.tile_pool(name="mpool", bufs=1) as mp, \
                 tc.tile_pool(name="wsp", bufs=2) as wsp, \
                 tc.tile_pool(name="psm", bufs=1, space="PSUM") as psm:

                # ---- routing ----
                cmr = tc.tile_pool(name="psr", bufs=1, space="PSUM")
                psr = cmr.__enter__()
                gw_its = []
                gws_its = []
                for sfx, sel_oh, cap, nit, base in (
                        ("A", selA, CAP_A, NIT_A, 0),
                        ("B", selB, CAP_B, NIT_B, CAP_A)):
                    wdcol = mp.tile([128, NTT], FP, tag=f"wdcol{sfx}",
                                    name=f"wdcol{sfx}")
                    for tt in range(NTT):
                        tsel = mp.tile([128, E], FP, tag="r_tsel", name="r_tsel")
                        nc.vector.tensor_tensor(out=tsel[:], in0=wd_sb[:, tt, :],
                                                in1=sel_oh[:], op=ALU.mult)
                        nc.vector.tensor_reduce(wdcol[:, tt:tt + 1], tsel[:],
                                                AX.X, ALU.add)
                    mask = mp.tile([128, NTT], FP, tag=f"mask{sfx}",
                                   name=f"mask{sfx}")
                    nc.vector.tensor_scalar(mask[:], wdcol[:], 0.0, None,
                                            ALU.is_gt)
                    ps_tot = psr.tile([8, 1], FP, tag="ps_ri", name="ps_tot")
                    nc.tensor.matmul(ps_tot[:], mask[:], ones_col[:],
                                     start=True, stop=True)
                    tot = mp.tile([8, 1], FP, tag="r_tot", name="r_tot")
                    nc.vector.tensor_copy(out=tot[:], in_=ps_tot[:])
                    rhs8 = mp.tile([8, 8], FP, tag="r_rhs8", name="r_rhs8")
                    nc.vector.tensor_scalar_mul(rhs8[:], tri_s8[:], tot[:])
                    ps_cum = psr.tile([128, NTT], FP, tag="ps_ri", name="ps_cum")
                    nc.tensor.matmul(ps_cum[:], tri_incl[:], mask[:],
                                     start=True, stop=False)
                    nc.tensor.matmul(ps_cum[:], ones8[:], rhs8[:],
                                     start=False, stop=True)
                    pos = mp.tile([128, NTT], FP, tag="r_pos", name="r_pos")
                    nc.scalar.activation(pos[:], ps_cum[:], AF.Copy, bias=-1.0)
                    posm = mp.tile([128, NTT], FP, tag="r_posm", name="r_posm")
                    nc.vector.scalar_tensor_tensor(posm[:], pos[:], 5.0,
                                                   mask[:], ALU.add, ALU.mult)
                    nc.vector.tensor_scalar_add(posm[:], posm[:], -5.0)

                    chunks = [(0, 512), (512, cap)] if cap > 512 else [(0, cap)]
                    ps_l2s = [psr.tile([2, hi - lo], FP, tag="ps_l2", bufs=2,
                                       name=f"ps_l2_{sfx}{ci}")
                              for ci, (lo, hi) in enumerate(chunks)]
                    for tt in range(NTT):
                        g_t = mp.tile([128, cap], FP, tag=f"r_g{sfx}",
                                      name=f"r_g{sfx}")
                        nc.vector.tensor_scalar(g_t[:], iota_bc[:, :cap],
                                                posm[:, tt:tt + 1], None,
                                                ALU.is_equal)
                        rhs2 = mp.tile([128, 2], FP, tag="r_rhs2", name="r_rhs2")
                        nc.vector.tensor_copy(out=rhs2[:, 0:1],
                                              in_=iota_t[:, tt:tt + 1])
                        nc.vector.tensor_copy(out=rhs2[:, 1:2],
                                              in_=wdcol[:, tt:tt + 1])
                        for ci, (lo, hi) in enumerate(chunks):
                            nc.tensor.matmul(ps_l2s[ci][:], rhs2[:],
                                             g_t[:, lo:hi],
                                             start=(tt == 0),
                                             stop=(tt == NTT - 1))
                    lg2 = mp.tile([2, cap], FP, tag=f"r_lg2{sfx}",
                                  name=f"r_lg2{sfx}")
                    for ci, (lo, hi) in enumerate(chunks):
                        nc.vector.tensor_copy(out=lg2[:, lo:hi],
                                              in_=ps_l2s[ci][:])
                    for it in range(nit):
                        pslt = psr.tile([128, 2], FP, tag="ps_lgT", name="pslt")
                        nc.tensor.transpose(pslt[:],
                                            lg2[:, it * 128:(it + 1) * 128],
                                            ident_f[:2, :2])
                        lgit = mp.tile([128, 2], FP, tag=f"r_lgit{sfx}{it}",
                                       name=f"r_lgit{sfx}{it}")
                        nc.vector.tensor_copy(out=lgit[:], in_=pslt[:])
                        gw_its.append(lgit)
                        gws = mp.tile([128, 1], FP, tag=f"r_gws{sfx}{it}",
                                      name=f"r_gws{sfx}{it}")
                        nc.vector.tensor_scalar_mul(gws[:], lgit[:, 1:2],
                                                    1.0 / WS)
                        gws_its.append(gws)
                        i16 = mp.tile([128, 1], I16, tag="r_i16", name="r_i16")
                        nc.vector.tensor_copy(out=i16[:], in_=lgit[:, 0:1])
                        off = base + it * 128
                        nc.sync.dma_start(out=d_idx[off:off + 128], in_=i16[:])

                idx_sb = mp.tile([128, CAP // 16], I16, tag="idx_sb")
                for r in range(8):
                    nc.sync.dma_start(
                        out=idx_sb[16 * r:16 * (r + 1), :],
                        in_=d_idx[:].rearrange("(c q) -> q c", q=16))
                cmr.__exit__(None, None, None)

                XeT_A = mp.tile([128, NDT, CAP_A], F8, tag="XeT_A")
                nc.gpsimd.dma_gather(
                    out_ap=XeT_A[:], in_ap=d_x3n[:],
                    idxs_ap=idx_sb[:, :CAP_A // 16],
                    num_idxs=CAP_A, num_idxs_reg=CAP_A, elem_size=D,
                    transpose=True, queue_num=0)
                XeT_B = mp.tile([128, NDT, CAP_B], F8, tag="XeT_B")
                nc.gpsimd.dma_gather(
                    out_ap=XeT_B[:], in_ap=d_x3n[:],
                    idxs_ap=idx_sb[:, CAP_A // 16:],
                    num_idxs=CAP_B, num_idxs_reg=CAP_B, elem_size=D,
                    transpose=True, queue_num=1)
                XeTs = {"A": XeT_A, "B": XeT_B}

                # ---- shared expert (initializes d_yp0/d_yp1) ----
                hs = mp.tile([128, NFT_S, T], BF, tag="hs")
                for ft in range(NFT_S):
                    ws1_t = wsp.tile([128, NDT, 128], BF, tag="w1t",
                                     name="ws1_t")
                    nc.sync.dma_start(out=ws1_t[:], in_=t["ws1"][ft])
                    ws3_t = wsp.tile([128, NDT, 128], BF, tag="w3t",
                                     name="ws3_t")
                    nc.sync.dma_start(out=ws3_t[:], in_=t["ws3"][ft])
                    for c in range(2):
                        sl = slice(c * 512, (c + 1) * 512)
                        ph1 = psm.tile([128, 512], FP, tag="ps_h1", name="ph1")
                        ph3 = psm.tile([128, 512], FP, tag="ps_h3", name="ph3")
                        for dt in range(NDT):
                            nc.tensor.matmul(ph1[:], ws1_t[:, dt, :],
                                             x3T[:, dt, sl],
                                             start=(dt == 0), stop=False)
                            nc.tensor.matmul(ph3[:], ws3_t[:, dt, :],
                                             x3T[:, dt, sl],
                                             start=(dt == 0), stop=False)
                        b_s1 = wsp.tile([1, 128], BF, tag="b1t", name="b_s1")
                        nc.sync.dma_start(out=b_s1[:], in_=t["bs1"][0:1, ft, :])
                        b_s3 = wsp.tile([1, 128], BF, tag="b3t", name="b_s3")
                        nc.sync.dma_start(out=b_s3[:], in_=t["bs3"][0:1, ft, :])
                        nc.tensor.matmul(ph1[:], b_s1[:], ones_row[:, :512],
                                         start=False, stop=True)
                        nc.tensor.matmul(ph3[:], b_s3[:], ones_row[:, :512],
                                         start=False, stop=True)
                        sg = mp.tile([128, 512], BF, tag="sg", name="sg")
                        nc.scalar.activation(sg[:], ph1[:], AF.Sigmoid)
                        a_t = mp.tile([128, 512], BF, tag="a_t", name="a_t")
                        nc.vector.scalar_tensor_tensor(a_t[:], ph1[:], 1.0,
                                                       sg[:], ALU.mult,
                                                       ALU.mult)
                        nc.vector.tensor_tensor(out=hs[:, ft, sl], in0=a_t[:],
                                                in1=ph3[:], op=ALU.mult)
                ws2_sb = mp.tile([128, NFT_S, D], BF, tag="ws2_sb")
                nc.sync.dma_start(out=ws2_sb[:],
                                  in_=t["ws2"][:].rearrange("f p d -> p f d"))
                for tt in range(NTT):
                    ys = wsp.tile([128, D], BF, tag="ys", name="ys")
                    for dc in range(4):
                        sl = slice(dc * 512, (dc + 1) * 512)
                        pys = psm.tile([128, 512], FP, tag="ps_ys", name="pys")
                        for ft in range(NFT_S):
                            nc.tensor.matmul(
                                pys[:], hs[:, ft, tt * 128:(tt + 1) * 128],
                                ws2_sb[:, ft, sl],
                                start=(ft == 0), stop=False)
                        b_s2 = wsp.tile([1, 512], BF, tag="b2t", name="b_s2")
                        nc.sync.dma_start(out=b_s2[:], in_=t["bs2_8"][0:1, sl])
                        nc.tensor.matmul(pys[:], ones_row[:, :128],
                                         b_s2[:], start=False, stop=True)
                        nc.scalar.activation(ys[:, sl], pys[:], AF.Copy)
                    tsl = slice(tt * 128, (tt + 1) * 128)
                    nc.sync.dma_start(out=d_yp0[tsl, :], in_=ys[:, :HD])
                    nc.sync.dma_start(out=d_yp1[tsl, :], in_=ys[:, HD:])

                # ---- routed experts: h for both, then ye per D-half ----
                hTs = {}
                for sfx, cap in (("A", CAP_A), ("B", CAP_B)):
                    XeT = XeTs[sfx]
                    XeTv = XeT[:].rearrange("p a b -> p (a b)").rearrange(
                        "p (c n i) -> p c i n", c=8, i=2)
                    hT = mp.tile([128, NFT + 1, cap], F8, tag=f"hT{sfx}",
                                 name=f"hT{sfx}")
                    hTs[sfx] = hT
                    nc.vector.memset(hT[:, NFT, :], 0)
                    chunks = [(0, 512), (512, cap)] if cap > 512 else [(0, cap)]
                    for ft in range(NFT):
                        w1_t = wsp.tile([128, 8, 2, 128], F8, tag="w1t",
                                        name="w1_t")
                        nc.sync.dma_start(out=w1_t[:], in_=t[f"w1{sfx}"][ft])
                        w3_t = wsp.tile([128, 8, 2, 128], F8, tag="w3t",
                                        name="w3_t")
                        nc.sync.dma_start(out=w3_t[:], in_=t[f"w3{sfx}"][ft])
                        for (lo, hi) in chunks:
                            w = hi - lo
                            ph1 = psm.tile([128, 512], FP, tag="ps_h1",
                                           name="ph1")
                            ph3 = psm.tile([128, 512], FP, tag="ps_h3",
                                           name="ph3")
                            for c8 in range(8):
                                nc.tensor.matmul(
                                    ph1[:, :w], w1_t[:, c8, :, :],
                                    XeTv[:, c8, :, lo:hi],
                                    start=(c8 == 0), stop=False,
                                    perf_mode=PM.DoubleRow)
                                nc.tensor.matmul(
                                    ph3[:, :w], w3_t[:, c8, :, :],
                                    XeTv[:, c8, :, lo:hi],
                                    start=(c8 == 0), stop=False,
                                    perf_mode=PM.DoubleRow)
                            b_1 = wsp.tile([1, 128], BF, tag="b1t", name="b_1")
                            nc.sync.dma_start(out=b_1[:],
                                              in_=t[f"b1{sfx}"][0:1, ft, :])
                            b_3 = wsp.tile([1, 128], BF, tag="b3t", name="b_3")
                            nc.sync.dma_start(out=b_3[:],
                                              in_=t[f"b3{sfx}"][0:1, ft, :])
                            nc.tensor.matmul(ph1[:, :w], b_1[:],
                                             ones_row[:, :w],
                                             start=False, stop=True)
                            nc.tensor.matmul(ph3[:, :w], b_3[:],
                                             ones_row[:, :w],
                                             start=False, stop=True)
                            sg = mp.tile([128, 512], BF, tag="sg", name="sg")
                            nc.scalar.activation(sg[:, :w], ph1[:, :w],
                                                 AF.Sigmoid, scale=1.0 / WS)
                            a_t = mp.tile([128, 512], BF, tag="a_t", name="a_t")
                            nc.vector.scalar_tensor_tensor(
                                a_t[:, :w], ph1[:, :w], 1.0 / WS, sg[:, :w],
                                ALU.mult, ALU.mult)
                            nc.vector.scalar_tensor_tensor(
                                hT[:, ft, lo:hi], ph3[:, :w], 1.0 / WS,
                                a_t[:, :w], ALU.mult, ALU.mult)

                cmy = tc.tile_pool(name="psy", bufs=1, space="PSUM")
                psy = cmy.__enter__()
                for half, d_yph, dcs in ((0, d_yp0, (0, 1)), (1, d_yp1, (2, 3))):
                    for sfx, cap, nit, base, it_base in (
                            ("A", CAP_A, NIT_A, 0, 0),
                            ("B", CAP_B, NIT_B, CAP_A, NIT_A)):
                        hT = hTs[sfx]
                        ye = mp.tile([128, nit, HD], BF, tag="ye",
                                     name=f"ye{sfx}{half}")
                        for dci, dc in enumerate(dcs):
                            sl = slice(dc * 512, (dc + 1) * 512)
                            osl = slice(dci * 512, (dci + 1) * 512)
                            pyes = [psy.tile([128, 512], FP, tag=f"ps_ye{i}",
                                             name=f"ps_ye_{sfx}{dc}_{i}")
                                    for i in range(nit)]
                            for ftp in range(NFTP):
                                w2_t = wsp.tile([128, 2, 512], F8, tag="w2t",
                                                name="w2_t")
                                nc.sync.dma_start(out=w2_t[:],
                                                  in_=t[f"w2{sfx}"][ftp, :, :, sl])
                                for it in range(nit):
                                    nc.tensor.matmul(
                                        pyes[it][:],
                                        hT[:, 2 * ftp:2 * ftp + 2,
                                           it * 128:(it + 1) * 128],
                                        w2_t[:], start=(ftp == 0), stop=False,
                                        perf_mode=PM.DoubleRow)
                            b_2 = wsp.tile([1, 512], BF, tag="b2t", name="b_2")
                            nc.sync.dma_start(out=b_2[:],
                                              in_=t[f"b2{sfx}"][0:1, sl])
                            for it in range(nit):
                                nc.tensor.matmul(pyes[it][:], ones_row[:, :128],
                                                 b_2[:], start=False, stop=True)
                                nc.scalar.activation(
                                    ye[:, it, osl], pyes[it][:], AF.Copy,
                                    scale=gws_its[it_base + it][:])
                        nc.gpsimd.dma_scatter_add(
                            out_ap=d_yph[:], in_ap=ye[:],
                            idxs_ap=idx_sb[:, base // 16:(base + cap) // 16],
                            num_idxs=cap, num_idxs_reg=cap, elem_size=HD,
                            queue_num=(0 if sfx == "A" else 1))
                    d_rsh = d_rs0 if half == 0 else d_rs1
                    nc.gpsimd.collective_compute(
                        "ReduceScatter", ALU.add, replica_groups=rg,
                        ins=[(d_yp0 if half == 0 else d_yp1)[:]],
                        outs=[d_rsh[:]])
                cmy.__exit__(None, None, None)

                # ---- final: residual + output ----
                rs_sb = mp.tile([128, D], BF, tag="rs_sb")
                nc.sync.dma_start(out=rs_sb[:, :HD], in_=d_rs0[:])
                nc.sync.dma_start(out=rs_sb[:, HD:], in_=d_rs1[:])
                nc.vector.tensor_tensor(out=x2_mine[:], in0=rs_sb[:],
                                        in1=x2_mine[:], op=ALU.add)
                nc.sync.dma_start(out=out_ext[:], in_=x2_mine[:])

    nc.compile()
    return nc


# --------------------------------------------------------------------------
# host-side input prep
# --------------------------------------------------------------------------

def _tile_w1(w):
    nft = w.shape[1] // 128
    return np.ascontiguousarray(
        w.reshape(NDT, 128, nft, 128).transpose(2, 1, 0, 3))


def _tile_w1_fp8(w):
    # [D, F] -> [NFT, 128(p), 8(c), 2(i), 128(f)] with d = c*256 + 2p + i
    return np.ascontiguousarray(
        (w * WS).astype(fp8).reshape(8, 128, 2, NFT, 128)
        .transpose(3, 1, 0, 2, 4))


def _tile_w2_fp8(w):
    # [F, D] -> [NFTP, 128(p), 2(i), D] with f = (2*ftp + i)*128 + p, pad to 12
    wp = np.zeros((NFTP * 2 * 128, D), np.float32)
    wp[:F] = w
    return np.ascontiguousarray(
        (wp * WS).astype(fp8).reshape(NFTP, 2, 128, D).transpose(0, 2, 1, 3))


def _prep_in_maps(inputs):
    f32 = lambda a: np.ascontiguousarray(np.asarray(a, dtype=np.float32))
    tobf = lambda a: np.ascontiguousarray(np.asarray(a, dtype=np.float32)
                                          .astype(bf16))
    x = f32(inputs["x"]).reshape(T, D)
    mask = f32(inputs["mask"])
    wq_w, wq_b = f32(inputs["wq_w"]), f32(inputs["wq_b"])
    wk_w, wk_b = f32(inputs["wk_w"]), f32(inputs["wk_b"])
    wv_w, wv_b = f32(inputs["wv_w"]), f32(inputs["wv_b"])
    wo_w, wo_b = f32(inputs["wo_w"]), f32(inputs["wo_b"])
    attn_g, ffn_g = f32(inputs["attn_g"]), f32(inputs["ffn_g"])
    gate_w, gate_b = f32(inputs["gate_w"]), f32(inputs["gate_b"])
    e_w1, e_b1 = f32(inputs["e_w1"]), f32(inputs["e_b1"])
    e_w2, e_b2 = f32(inputs["e_w2"]), f32(inputs["e_b2"])
    e_w3, e_b3 = f32(inputs["e_w3"]), f32(inputs["e_b3"])
    s_w1, s_b1 = f32(inputs["s_w1"]), f32(inputs["s_b1"])
    s_w2, s_b2 = f32(inputs["s_w2"]), f32(inputs["s_b2"])
    s_w3, s_b3 = f32(inputs["s_w3"]), f32(inputs["s_b3"])

    s_w1p = np.zeros((D, FS_PAD), np.float32); s_w1p[:, :FS] = s_w1
    s_w3p = np.zeros((D, FS_PAD), np.float32); s_w3p[:, :FS] = s_w3
    s_b1p = np.zeros(FS_PAD, np.float32); s_b1p[:FS] = s_b1
    s_b3p = np.zeros(FS_PAD, np.float32); s_b3p[:FS] = s_b3
    s_w2p = np.zeros((FS_PAD, D), np.float32); s_w2p[:FS] = s_w2

    i_idx = np.arange(T)[:, None]
    j_idx = np.arange(T)[None, :]
    rel = np.where(i_idx >= j_idx, -(i_idx - j_idx).astype(np.float32), 0.0)
    ident = np.eye(128, dtype=np.float32)
    tri_incl = (np.arange(128)[:, None] <= np.arange(128)[None, :]) \
        .astype(np.float32)
    tri_s8 = (np.arange(8)[:, None] < np.arange(8)[None, :]).astype(np.float32)
    iota_bc = np.tile(np.arange(CAP_A, dtype=np.float32), (128, 1))
    iota_t = (np.arange(NTT)[None, :] * 128
              + np.arange(128)[:, None]).astype(np.float32)

    in_maps = []
    for c in range(NCORES):
        eA, eB = A_EXPERTS[c], B_EXPERTS[c]
        slope = 2.0 ** (-(c + 1))
        selA = np.zeros(E, np.float32); selA[eA] = 1.0
        selB = np.zeros(E, np.float32); selB[eB] = 1.0
        ssel = np.zeros(NTT, np.float32); ssel[c] = 1.0
        fs_lo = c * FS_SLICE
        fs_hi = fs_lo + FS_SLICE
        m = {
            "x_full": x,
            "am": (mask + slope * rel).astype(np.float32),
            "wq": tobf(wq_w[:, c * DK:(c + 1) * DK]),
            "wk": tobf(wk_w[:, c * DK:(c + 1) * DK]),
            "wv": tobf(wv_w[:, c * DV:(c + 1) * DV]),
            "bq": tobf(wq_b[c * DK:(c + 1) * DK]).reshape(1, DK),
            "bk": tobf(wk_b[c * DK:(c + 1) * DK]).reshape(1, DK),
            "bv": tobf(wv_b[c * DV:(c + 1) * DV]).reshape(1, DV),
            "wo": tobf(wo_w),
            "wo_b": tobf(wo_b).reshape(1, D),
            "g1": np.tile(attn_g, (128, 1)),
            "g2": np.tile(ffn_g, (128, 1)),
            "gate_wT": np.ascontiguousarray(gate_w.T),
            "gate_b3": np.tile(gate_b, (128, NTT, 1)).astype(np.float32),
            "selA": np.tile(selA, (128, 1)),
            "selB": np.tile(selB, (128, 1)),
            "slice_sel": np.tile(ssel, (128, 1)),
            "ws1": _tile_w1(tobf(s_w1p[:, fs_lo:fs_hi])),
            "ws3": _tile_w1(tobf(s_w3p[:, fs_lo:fs_hi])),
            "ws2": tobf(s_w2p[fs_lo:fs_hi]).reshape(NFT_S, 128, D),
            "bs1": tobf(s_b1p[fs_lo:fs_hi]).reshape(1, NFT_S, 128),
            "bs3": tobf(s_b3p[fs_lo:fs_hi]).reshape(1, NFT_S, 128),
            "bs2_8": tobf(s_b2 / 8.0).reshape(1, D),
            "ident": ident.astype(bf16),
            "ident_f": ident,
            "tri_incl": tri_incl,
            "tri_s8": tri_s8,
            "ones8": np.ones((8, 128), np.float32),
            "ones_col": np.ones((128, 1), np.float32),
            "iota_bc": iota_bc,
            "iota_t": iota_t,
            "ones_row": np.ones((1, 1024), bf16),
        }
        for sfx, e in (("A", eA), ("B", eB)):
            m[f"w1{sfx}"] = _tile_w1_fp8(e_w1[e])
            m[f"w3{sfx}"] = _tile_w1_fp8(e_w3[e])
            m[f"w2{sfx}"] = _tile_w2_fp8(e_w2[e])
            m[f"b1{sfx}"] = tobf(e_b1[e] * WS).reshape(1, NFT, 128)
            m[f"b3{sfx}"] = tobf(e_b3[e] * WS).reshape(1, NFT, 128)
            m[f"b2{sfx}"] = tobf(e_b2[e] * WS).reshape(1, D)
        in_maps.append(m)
    return in_maps


def _get_nc():
    if "nc" not in _CACHE:
        _CACHE["nc"] = _build_nc()
    return _CACHE["nc"]


def kernel(trace=False, **inputs):
    nc = _get_nc()
    in_maps = _prep_in_maps(inputs)
    res = run_bass_kernel_spmd(nc, in_maps, core_ids=list(range(NCORES)),
                               trace=trace)
    out = np.concatenate([res.results[c]["out"] for c in range(NCORES)],
                         axis=0).reshape(1, T, D).astype(np.float32)
    if trace:
        return out, res
    return out


# revision 17
# speedup vs baseline: 1.1932x; 1.1272x over previous
"""Distributed Trainium2 Bass kernel for nn_Block_32332513804635 (moe_routing).

Transformer block: LN -> 8-head attention (alibi+causal) -> residual -> LN ->
MoE (16 routed experts, top-6, SwiGLU) + shared expert -> residual.

Sharding over 8 NeuronCores (SPMD, one graph; per-core differences via data):
  - LN1/LN2/gating/attention out-projection: replicated over full tokens
    (streamed per 128-token tile) -- trades idle-engine compute for the
    removal of three collective barriers.
  - attention: head-parallel (1 head/core), AllGather of per-head outputs.
  - routed experts: expert-parallel, 2 experts/core (cap 640 "big" + cap 384
    "small", pairing balances measured loads); on-device top-6 routing,
    dispatch via dma_gather(transpose), combine via gating-scaled
    dma_scatter_add into bf16 partial buffers split in two D-halves.
  - shared expert: intermediate-dim-parallel (2816 padded to 3072 = 8*384).
  - final: two ReduceScatters (one per D-half, first overlaps expert
    compute), residual selected via host-fed one-hot accumulate.

kernel(**inputs) takes FULL unsharded inputs, returns the FULL output.
"""
import numpy as np
import ml_dtypes

import concourse.bacc as bacc
import concourse.tile as tile
import concourse.mybir as mybir
import concourse.library_config as library_config
from concourse.bass_utils import run_bass_kernel_spmd

BF = mybir.dt.bfloat16
FP = mybir.dt.float32
I16 = mybir.dt.int16
F8 = mybir.dt.float8e4
PM = mybir.MatmulPerfMode
AF = mybir.ActivationFunctionType
ALU = mybir.AluOpType
AX = mybir.AxisListType

bf16 = ml_dtypes.bfloat16
fp8 = mybir.dt.np(mybir.dt.float8e4)
WS = 1024.0            # fp8 weight scale
NFTP = 6               # ft pairs for DoubleRow ye (11 tiles + 1 zero pad)

NCORES = 8
T, D = 1024, 2048
HD = D // 2                          # D-half for the split combine
H, DK, DV = 8, 128, 128
E, TOPK, F = 16, 6, 1408
FS, FS_PAD = 2816, 3072
FS_SLICE = FS_PAD // NCORES          # 384
NFT_S = FS_SLICE // 128              # 3
NFT = F // 128                       # 11
NDT = D // 128                       # 16
NTT = T // 128                       # 8
TS = T // NCORES                     # 128
CAP_A, CAP_B = 640, 384
CAP = CAP_A + CAP_B
NIT_A, NIT_B = CAP_A // 128, CAP_B // 128
EPS = 1e-8
A_EXPERTS = [3, 5, 13, 0, 4, 9, 12, 14]
B_EXPERTS = [10, 11, 15, 1, 2, 6, 7, 8]

_CACHE = {}

_INPUT_SPECS = [
    ("x_full", [T, D], FP), ("am", [T, T], FP),
    ("wq", [D, DK], BF), ("wk", [D, DK], BF), ("wv", [D, DV], BF),
    ("bq", [1, DK], BF), ("bk", [1, DK], BF), ("bv", [1, DV], BF),
    ("wo", [H * DV, D], BF), ("wo_b", [1, D], BF),
    ("g1", [128, D], FP), ("g2", [128, D], FP),
    ("gate_wT", [D, E], FP), ("gate_b3", [128, NTT, E], FP),
    ("selA3", [128, NTT, E], FP), ("selB3", [128, NTT, E], FP),
    ("slice_sel", [128, NTT], FP),
    ("w1A", [NFT, 128, 8, 2, 128], F8), ("w3A", [NFT, 128, 8, 2, 128], F8),
    ("w2A", [NFTP, 128, 2, D], F8),
    ("b1A", [128, NFT], FP), ("b3A", [128, NFT], FP), ("b2A", [1, D], BF),
    ("w1B", [NFT, 128, 8, 2, 128], F8), ("w3B", [NFT, 128, 8, 2, 128], F8),
    ("w2B", [NFTP, 128, 2, D], F8),
    ("b1B", [128, NFT], FP), ("b3B", [128, NFT], FP), ("b2B", [1, D], BF),
    ("ws1", [NFT_S, 128, NDT, 128], BF), ("ws3", [NFT_S, 128, NDT, 128], BF),
    ("ws2", [NFT_S, 128, D], BF),
    ("bs1", [128, NFT_S], FP), ("bs3", [128, NFT_S], FP),
    ("bs2_8", [1, D], BF),
    ("ident", [128, 128], BF), ("ident_f", [128, 128], FP),
    ("tri_incl", [128, 128], FP), ("tri_s8", [8, 8], FP),
    ("ones8", [8, 128], FP), ("ones_col", [128, 1], FP),
    ("iota_bc", [128, CAP_A], FP), ("iota_t", [128, NTT], FP),
    ("ones_row", [1, 1024], BF),
]


def _build_nc():
    nc = bacc.Bacc("TRN2", target_bir_lowering=False, debug=False,
                   num_devices=NCORES, num_swdge_queues=2)
    t = {}
    for name, shape, dt in _INPUT_SPECS:
        t[name] = nc.dram_tensor(name, list(shape), dt, kind="ExternalInput")
    out_ext = nc.dram_tensor("out", [TS, D], FP, kind="ExternalOutput")

    d_oTs = [nc.dram_tensor(f"d_oT{c}", [DV, 512], BF) for c in range(2)]
    ag_oTs = [nc.dram_tensor(f"ag_oT{c}", [H * DV, 512], BF,
                             addr_space="Shared") for c in range(2)]
    d_x3n = nc.dram_tensor("d_x3n", [T, D], F8)
    d_idx = nc.dram_tensor("d_idx", [CAP], I16)
    d_yp0 = nc.dram_tensor("d_yp0", [T, HD], BF)
    d_yp1 = nc.dram_tensor("d_yp1", [T, HD], BF)
    d_rs0 = nc.dram_tensor("d_rs0", [TS, HD], BF)
    d_rs1 = nc.dram_tensor("d_rs1", [TS, HD], BF)

    rg = [list(range(NCORES))]

    with tile.TileContext(nc) as tc:
        with tc.tile_pool(name="cpool", bufs=1) as cp, \
             tc.tile_pool(name="ppool", bufs=1) as pp:

            nc.gpsimd.load_library(library_config.mlp)

            def load(pool, name):
                src = t[name]
                tl = pool.tile(list(src.shape), src.dtype, tag=name, name=name)
                nc.sync.dma_start(out=tl[:], in_=src[:])
                return tl

            ident = load(cp, "ident")
            ident_f = load(cp, "ident_f")
            tri_incl = load(cp, "tri_incl")
            tri_s8 = load(cp, "tri_s8")
            ones8 = load(cp, "ones8")
            ones_col = load(cp, "ones_col")
            iota_bc = load(cp, "iota_bc")
            iota_t = load(cp, "iota_t")
            ones_row = load(cp, "ones_row")
            g1 = load(cp, "g1")
            g2 = load(cp, "g2")
            gate_b3 = load(cp, "gate_b3")
            selA3 = load(cp, "selA3")
            selB3 = load(cp, "selB3")
            slice_sel = load(cp, "slice_sel")
            gate_w_sb = cp.tile([128, NDT, E], FP, tag="gate_w")
            nc.sync.dma_start(out=gate_w_sb[:], in_=t["gate_wT"][:].rearrange(
                "(dt p) e -> p dt e", p=128))

            x2_mine = pp.tile([128, D], FP, tag="x2_mine")
            nc.vector.memset(x2_mine[:], 0.0)
            wd_sb = pp.tile([128, NTT, E], FP, tag="wd")
            x3T = pp.tile([128, NDT, T], BF, tag="x3T")

            def layer_norm(pool, src, gb, dst):
                s = pool.tile([128, 1], FP, tag="ln_s", name="ln_s")
                nc.vector.tensor_reduce(s[:], src[:], AX.X, ALU.add)
                negmu = pool.tile([128, 1], FP, tag="ln_negmu", name="ln_negmu")
                nc.vector.tensor_scalar_mul(negmu[:], s[:], -1.0 / D)
                sq = pool.tile([128, D], FP, tag="ln_tmp", bufs=1, name="ln_sq")
                ssq = pool.tile([128, 1], FP, tag="ln_ssq", name="ln_ssq")
                nc.scalar.activation(sq[:], src[:], AF.Square,
                                     bias=negmu[:], accum_out=ssq[:])
                var = pool.tile([128, 1], FP, tag="ln_var", name="ln_var")
                nc.vector.tensor_scalar(var[:], ssq[:], 1.0 / D, EPS,
                                        ALU.mult, ALU.add)
                sd = pool.tile([128, 1], FP, tag="ln_sd", name="ln_sd")
                nc.scalar.activation(sd[:], var[:], AF.Sqrt)
                rstd = pool.tile([128, 1], FP, tag="ln_rstd", name="ln_rstd")
                nc.vector.reciprocal(rstd[:], sd[:])
                tmp = pool.tile([128, D], FP, tag="ln_tmp", bufs=1, name="ln_tmp")
                nc.vector.scalar_tensor_tensor(tmp[:], src[:], negmu[:],
                                               gb[:], ALU.add, ALU.mult)
                nc.vector.tensor_scalar_mul(dst[:], tmp[:], rstd[:])

            # =============================================================
            # Phase 1+2: LN1 (all tokens, streamed) + attention head
            # =============================================================
            with tc.tile_pool(name="apool", bufs=1) as ap, \
                 tc.tile_pool(name="amp", bufs=2) as amp, \
                 tc.tile_pool(name="pst", bufs=2, space="PSUM") as pst:

                x1T = ap.tile([128, NDT, T], BF, tag="x1T")
                for tt in range(NTT):
                    x_t = amp.tile([128, D], FP, tag="x_t", name="x_t")
                    nc.sync.dma_start(out=x_t[:],
                                      in_=t["x_full"][tt * 128:(tt + 1) * 128, :])
                    x1 = amp.tile([128, D], FP, tag="x1", name="x1")
                    layer_norm(amp, x_t, g1, x1)
                    x1b = amp.tile([128, D], BF, tag="x1b", name="x1b")
                    nc.vector.tensor_copy(out=x1b[:], in_=x1[:])
                    for dt in range(NDT):
                        pt = pst.tile([128, 128], BF, tag="ps_tr", name="pt")
                        nc.tensor.transpose(
                            pt[:], x1b[:, dt * 128:(dt + 1) * 128], ident[:])
                        nc.vector.tensor_copy(
                            out=x1T[:, dt, tt * 128:(tt + 1) * 128], in_=pt[:])

                # ---- attention head ----
                cm2 = tc.tile_pool(name="ps2", bufs=2, space="PSUM")
                ps2 = cm2.__enter__()
                wq_sb = ap.tile([128, NDT, DK], BF, tag="wq")
                nc.sync.dma_start(out=wq_sb[:], in_=t["wq"][:].rearrange(
                    "(dt p) f -> p dt f", p=128))
                wk_sb = ap.tile([128, NDT, DK], BF, tag="wk")
                nc.sync.dma_start(out=wk_sb[:], in_=t["wk"][:].rearrange(
                    "(dt p) f -> p dt f", p=128))
                wv_sb = ap.tile([128, NDT, DV], BF, tag="wv")
                nc.sync.dma_start(out=wv_sb[:], in_=t["wv"][:].rearrange(
                    "(dt p) f -> p dt f", p=128))
                bq = load(ap, "bq")
                bk = load(ap, "bk")
                bv = load(ap, "bv")

                qT = ap.tile([128, T], BF, tag="qT")
                kT = ap.tile([128, T], BF, tag="kT")
                for dst, w_sb, b_sb in ((qT, wq_sb, bq), (kT, wk_sb, bk)):
                    for c in range(2):
                        sl = slice(c * 512, (c + 1) * 512)
                        ps = ps2.tile([128, 512], FP, tag="ps_qk", name="ps_qk")
                        for dt in range(NDT):
                            nc.tensor.matmul(ps[:], w_sb[:, dt, :],
                                             x1T[:, dt, sl],
                                             start=(dt == 0), stop=False)
                        nc.tensor.matmul(ps[:], b_sb[:], ones_row[:, :512],
                                         start=False, stop=True)
                        nc.scalar.activation(dst[:, sl], ps[:], AF.Copy)

                v_sb = ap.tile([128, NTT, DV], BF, tag="v_sb")
                for tt in range(NTT):
                    ps = ps2.tile([128, DV], FP, tag="ps_v", name="ps_v")
                    for dt in range(NDT):
                        nc.tensor.matmul(
                            ps[:], x1T[:, dt, tt * 128:(tt + 1) * 128],
                            wv_sb[:, dt, :], start=(dt == 0), stop=False)
                    nc.tensor.matmul(ps[:], ones_row[:, :128], bv[:],
                                     start=False, stop=True)
                    nc.scalar.activation(v_sb[:, tt, :], ps[:], AF.Copy)

                cm2.__exit__(None, None, None)
                cm3 = tc.tile_pool(name="ps3", bufs=2, space="PSUM")
                ps3 = cm3.__enter__()
                p_sb = ap.tile([128, NTT, T], BF, tag="p_sb")
                for tt in range(NTT):
                    am_t = amp.tile([128, T], FP, tag="am_t", name="am_t")
                    nc.sync.dma_start(out=am_t[:],
                                      in_=t["am"][tt * 128:(tt + 1) * 128, :])
                    s_sb = amp.tile([128, T], FP, tag="s_sb", name="s_sb")
                    for c in range(2):
                        sl = slice(c * 512, (c + 1) * 512)
                        ps = ps3.tile([128, 512], FP, tag="ps_s", name="ps_s")
                        nc.tensor.matmul(ps[:], qT[:, tt * 128:(tt + 1) * 128],
                                         kT[:, sl], start=True, stop=True)
                        nc.vector.scalar_tensor_tensor(
                            s_sb[:, sl], ps[:], DK ** -0.5, am_t[:, sl],
                            ALU.mult, ALU.add)
                    negmax = amp.tile([128, 1], FP, tag="negmax", name="negmax")
                    nc.vector.tensor_reduce(negmax[:], s_sb[:], AX.X, ALU.max,
                                            negate=True)
                    sumexp = amp.tile([128, 1], FP, tag="sumexp", name="sumexp")
                    nc.scalar.activation(p_sb[:, tt, :], s_sb[:], AF.Exp,
                                         bias=negmax[:], accum_out=sumexp[:])
                    rec = amp.tile([128, 1], FP, tag="rec", name="rec")
                    nc.vector.reciprocal(rec[:], sumexp[:])
                    nc.vector.tensor_scalar_mul(v_sb[:, tt, :], v_sb[:, tt, :],
                                                rec[:])

                oT = ap.tile([128, T], BF, tag="oT")
                for c in range(2):
                    sl = slice(c * 512, (c + 1) * 512)
                    ps = ps3.tile([128, 512], FP, tag="ps_o", name="ps_o")
                    for tt in range(NTT):
                        nc.tensor.matmul(ps[:], v_sb[:, tt, :], p_sb[:, tt, sl],
                                         start=(tt == 0), stop=(tt == NTT - 1))
                    nc.scalar.activation(oT[:, sl], ps[:], AF.Copy)
                    nc.sync.dma_start(out=d_oTs[c][:], in_=oT[:, sl])
                    nc.gpsimd.collective_compute(
                        "AllGather", ALU.bypass, replica_groups=rg,
                        ins=[d_oTs[c][:]], outs=[ag_oTs[c][:]])
                cm3.__exit__(None, None, None)

            # =============================================================
            # Phase 3: out-projection + x2 + LN2 + gating (all tokens)
            # =============================================================
            with tc.tile_pool(name="bpool", bufs=1) as bp, \
                 tc.tile_pool(name="bmp", bufs=2) as bmp, \
                 tc.tile_pool(name="ps4", bufs=1, space="PSUM") as ps4:

                oT_all = bp.tile([128, H, T], BF, tag="oT_all")
                for c in range(2):
                    nc.sync.dma_start(
                        out=oT_all[:, :, c * 512:(c + 1) * 512],
                        in_=ag_oTs[c][:].rearrange("(ht p) s -> p ht s", p=128))
                wo_sb = bp.tile([128, H, D], BF, tag="wo_sb")
                nc.sync.dma_start(out=wo_sb[:], in_=t["wo"][:].rearrange(
                    "(ht p) d -> p ht d", p=128))
                wo_b = load(bp, "wo_b")
                logits = bp.tile([128, NTT, E], FP, tag="logits")

                for tt in range(NTT):
                    tsl = slice(tt * 128, (tt + 1) * 128)
                    x2_t = bmp.tile([128, D], FP, tag="x2_t", name="x2_t")
                    x_t2 = bmp.tile([128, D], FP, tag="x_t2", bufs=1, name="x_t2")
                    nc.sync.dma_start(out=x_t2[:], in_=t["x_full"][tsl, :])
                    for dc in range(4):
                        sl = slice(dc * 512, (dc + 1) * 512)
                        ps = ps4.tile([128, 512], FP, tag=f"ps_x2{dc % 2}",
                                      bufs=2, name="ps_x2")
                        for ht in range(H):
                            nc.tensor.matmul(ps[:], oT_all[:, ht, tsl],
                                             wo_sb[:, ht, sl],
                                             start=(ht == 0), stop=False)
                        nc.tensor.matmul(ps[:], ones_row[:, :128],
                                         wo_b[:, sl], start=False, stop=True)
                        nc.vector.tensor_tensor(out=x2_t[:, sl], in0=ps[:],
                                                in1=x_t2[:, sl], op=ALU.add)
                    nc.vector.scalar_tensor_tensor(
                        x2_mine[:], x2_t[:], slice_sel[:, tt:tt + 1],
                        x2_mine[:], ALU.mult, ALU.add)
                    x3 = bmp.tile([128, D], FP, tag="x3", name="x3")
                    layer_norm(bmp, x2_t, g2, x3)
                    x3b = bmp.tile([128, D], F8, tag="x3b", bufs=1, name="x3b")
                    nc.vector.tensor_copy(out=x3b[:], in_=x3[:])
                    nc.sync.dma_start(out=d_x3n[tsl, :], in_=x3b[:])
                    x3Tf = bmp.tile([128, NDT, 128], FP, tag="x3Tf", bufs=1,
                                    name="x3Tf")
                    for dt in range(NDT):
                        ptf = ps4.tile([128, 128], FP, tag="ps_tr", bufs=2,
                                       name="ptf")
                        nc.tensor.transpose(
                            ptf[:], x3[:, dt * 128:(dt + 1) * 128], ident_f[:])
                        nc.vector.tensor_copy(out=x3Tf[:, dt, :], in_=ptf[:])
                    nc.vector.tensor_copy(out=x3T[:, :, tsl], in_=x3Tf[:])
                    ps_lg = ps4.tile([128, E], FP, tag="ps_lgt", bufs=2,
                                     name="ps_lg")
                    for dt in range(NDT):
                        nc.tensor.matmul(ps_lg[:], x3Tf[:, dt, :],
                                         gate_w_sb[:, dt, :],
                                         start=(dt == 0), stop=(dt == NDT - 1))
                    nc.vector.tensor_copy(out=logits[:, tt, :], in_=ps_lg[:])

                # batched softmax + top-6 over [128, NTT, E]
                mx = bp.tile([128, NTT], FP, tag="g_mx")
                nc.vector.tensor_reduce(mx[:], logits[:], AX.X, ALU.max)
                sh = bp.tile([128, NTT, E], FP, tag="g_sh")
                nc.vector.tensor_tensor(out=sh[:], in0=logits[:],
                                        in1=mx[:].broadcast_to([128, NTT, E]),
                                        op=ALU.subtract)
                ex = bp.tile([128, NTT, E], FP, tag="g_ex")
                nc.scalar.activation(ex[:], sh[:], AF.Exp)
                se = bp.tile([128, NTT], FP, tag="g_se")
                nc.vector.tensor_reduce(se[:], ex[:], AX.X, ALU.add)
                rec = bp.tile([128, NTT], FP, tag="g_rec")
                nc.vector.reciprocal(rec[:], se[:])
                sm = bp.tile([128, NTT, E], FP, tag="g_sm")
                nc.vector.tensor_tensor(out=sm[:], in0=ex[:],
                                        in1=rec[:].broadcast_to([128, NTT, E]),
                                        op=ALU.mult)
                cur = bp.tile([128, NTT, E], FP, tag="g_cur")
                nc.vector.tensor_tensor(out=cur[:], in0=sm[:], in1=gate_b3[:],
                                        op=ALU.add)
                nc.vector.memset(wd_sb[:], 0.0)
                for _ in range(TOPK):
                    mx2 = bp.tile([128, NTT], FP, tag="g_mx2", name="g_mx2")
                    nc.vector.tensor_reduce(mx2[:], cur[:], AX.X, ALU.max)
                    oh = bp.tile([128, NTT, E], FP, tag="g_oh", name="g_oh")
                    nc.vector.tensor_tensor(
                        out=oh[:], in0=cur[:],
                        in1=mx2[:].broadcast_to([128, NTT, E]), op=ALU.is_ge)
                    t1 = bp.tile([128, NTT, E], FP, tag="g_t1", name="g_t1")
                    nc.vector.tensor_tensor(out=t1[:], in0=oh[:], in1=sm[:],
                                            op=ALU.mult)
                    nc.vector.tensor_tensor(out=wd_sb[:], in0=wd_sb[:],
                                            in1=t1[:], op=ALU.add)
                    nc.vector.scalar_tensor_tensor(cur[:], oh[:], -1e30,
                                                   cur[:], ALU.mult, ALU.add)

            # =============================================================
            # Phase 4: routing, gather, shared expert, routed experts
            # =============================================================
            with tc.tile_pool(name="mpool", bufs=1) as mp, \
                 tc.tile_pool(name="wsp", bufs=2) as wsp, \
                 tc.tile_pool(name="psm", bufs=1, space="PSUM") as psm:

                # ---- routing ----
                cmr = tc.tile_pool(name="psr", bufs=1, space="PSUM")
                psr = cmr.__enter__()
                gw_its = []
                gws_its = []
                for sfx, sel_oh, cap, nit, base in (
                        ("A", selA3, CAP_A, NIT_A, 0),
                        ("B", selB3, CAP_B, NIT_B, CAP_A)):
                    tsel = mp.tile([128, NTT, E], FP, tag="r_tsel",
                                   name="r_tsel")
                    nc.vector.tensor_tensor(out=tsel[:], in0=wd_sb[:],
                                            in1=sel_oh[:], op=ALU.mult)
                    wdcol = mp.tile([128, NTT], FP, tag=f"wdcol{sfx}",
                                    name=f"wdcol{sfx}")
                    nc.vector.tensor_reduce(wdcol[:], tsel[:], AX.X, ALU.add)
                    mask = mp.tile([128, NTT], FP, tag=f"mask{sfx}",
                                   name=f"mask{sfx}")
                    nc.vector.tensor_scalar(mask[:], wdcol[:], 0.0, None,
                                            ALU.is_gt)
                    ps_tot = psr.tile([8, 1], FP, tag="ps_ri", name="ps_tot")
                    nc.tensor.matmul(ps_tot[:], mask[:], ones_col[:],
                                     start=True, stop=True)
                    tot = mp.tile([8, 1], FP, tag="r_tot", name="r_tot")
                    nc.vector.tensor_copy(out=tot[:], in_=ps_tot[:])
                    rhs8 = mp.tile([8, 8], FP, tag="r_rhs8", name="r_rhs8")
                    nc.vector.tensor_scalar_mul(rhs8[:], tri_s8[:], tot[:])
                    ps_cum = psr.tile([128, NTT], FP, tag="ps_ri", name="ps_cum")
                    nc.tensor.matmul(ps_cum[:], tri_incl[:], mask[:],
                                     start=True, stop=False)
                    nc.tensor.matmul(ps_cum[:], ones8[:], rhs8[:],
                                     start=False, stop=True)
                    pos = mp.tile([128, NTT], FP, tag="r_pos", name="r_pos")
                    nc.scalar.activation(pos[:], ps_cum[:], AF.Copy, bias=-1.0)
                    posm = mp.tile([128, NTT], FP, tag="r_posm", name="r_posm")
                    nc.vector.scalar_tensor_tensor(posm[:], pos[:], 5.0,
                                                   mask[:], ALU.add, ALU.mult)
                    nc.vector.tensor_scalar_add(posm[:], posm[:], -5.0)

                    chunks = [(0, 512), (512, cap)] if cap > 512 else [(0, cap)]
                    ps_l2s = [psr.tile([2, hi - lo], FP, tag="ps_l2", bufs=2,
                                       name=f"ps_l2_{sfx}{ci}")
                              for ci, (lo, hi) in enumerate(chunks)]
                    for tt in range(NTT):
                        g_t = mp.tile([128, cap], FP, tag=f"r_g{sfx}",
                                      name=f"r_g{sfx}")
                        nc.vector.tensor_scalar(g_t[:], iota_bc[:, :cap],
                                                posm[:, tt:tt + 1], None,
                                                ALU.is_equal)
                        rhs2 = mp.tile([128, 2], FP, tag="r_rhs2", name="r_rhs2")
                        nc.vector.tensor_copy(out=rhs2[:, 0:1],
                                              in_=iota_t[:, tt:tt + 1])
                        nc.vector.tensor_copy(out=rhs2[:, 1:2],
                                              in_=wdcol[:, tt:tt + 1])
                        for ci, (lo, hi) in enumerate(chunks):
                            nc.tensor.matmul(ps_l2s[ci][:], rhs2[:],
                                             g_t[:, lo:hi],
                                             start=(tt == 0),
                                             stop=(tt == NTT - 1))
                    lg2 = mp.tile([2, cap], FP, tag=f"r_lg2{sfx}",
                                  name=f"r_lg2{sfx}")
                    for ci, (lo, hi) in enumerate(chunks):
                        nc.vector.tensor_copy(out=lg2[:, lo:hi],
                                              in_=ps_l2s[ci][:])
                    for it in range(nit):
                        pslt = psr.tile([128, 2], FP, tag="ps_lgT", name="pslt")
                        nc.tensor.transpose(pslt[:],
                                            lg2[:, it * 128:(it + 1) * 128],
                                            ident_f[:2, :2])
                        lgit = mp.tile([128, 2], FP, tag=f"r_lgit{sfx}{it}",
                                       name=f"r_lgit{sfx}{it}")
                        nc.vector.tensor_copy(out=lgit[:], in_=pslt[:])
                        gw_its.append(lgit)
                        gws = mp.tile([128, 1], FP, tag=f"r_gws{sfx}{it}",
                                      name=f"r_gws{sfx}{it}")
                        nc.vector.tensor_scalar_mul(gws[:], lgit[:, 1:2],
                                                    1.0 / WS)
                        gws_its.append(gws)
                        i16 = mp.tile([128, 1], I16, tag="r_i16", name="r_i16")
                        nc.vector.tensor_copy(out=i16[:], in_=lgit[:, 0:1])
                        off = base + it * 128
                        nc.sync.dma_start(out=d_idx[off:off + 128], in_=i16[:])

                idx_sb = mp.tile([128, CAP // 16], I16, tag="idx_sb")
                for r in range(8):
                    nc.sync.dma_start(
                        out=idx_sb[16 * r:16 * (r + 1), :],
                        in_=d_idx[:].rearrange("(c q) -> q c", q=16))
                cmr.__exit__(None, None, None)

                XeT_A = mp.tile([128, NDT, CAP_A], F8, tag="XeT_A")
                nc.gpsimd.dma_gather(
                    out_ap=XeT_A[:], in_ap=d_x3n[:],
                    idxs_ap=idx_sb[:, :CAP_A // 16],
                    num_idxs=CAP_A, num_idxs_reg=CAP_A, elem_size=D,
                    transpose=True, queue_num=0)
                XeT_B = mp.tile([128, NDT, CAP_B], F8, tag="XeT_B")
                nc.gpsimd.dma_gather(
                    out_ap=XeT_B[:], in_ap=d_x3n[:],
                    idxs_ap=idx_sb[:, CAP_A // 16:],
                    num_idxs=CAP_B, num_idxs_reg=CAP_B, elem_size=D,
                    transpose=True, queue_num=1)
                XeTs = {"A": XeT_A, "B": XeT_B}

                # ---- shared expert (initializes d_yp0/d_yp1) ----
                bs1f = load(mp, "bs1")
                bs3f = load(mp, "bs3")
                hs = mp.tile([128, NFT_S, T], BF, tag="hs")
                for ft in range(NFT_S):
                    ws1_t = wsp.tile([128, NDT, 128], BF, tag="w1t",
                                     name="ws1_t")
                    nc.sync.dma_start(out=ws1_t[:], in_=t["ws1"][ft])
                    ws3_t = wsp.tile([128, NDT, 128], BF, tag="w3t",
                                     name="ws3_t")
                    nc.sync.dma_start(out=ws3_t[:], in_=t["ws3"][ft])
                    for c in range(2):
                        sl = slice(c * 512, (c + 1) * 512)
                        ph1 = psm.tile([128, 512], FP, tag="ps_h1", name="ph1")
                        ph3 = psm.tile([128, 512], FP, tag="ps_h3", name="ph3")
                        for dt in range(NDT):
                            nc.tensor.matmul(ph1[:], ws1_t[:, dt, :],
                                             x3T[:, dt, sl],
                                             start=(dt == 0),
                                             stop=(dt == NDT - 1))
                            nc.tensor.matmul(ph3[:], ws3_t[:, dt, :],
                                             x3T[:, dt, sl],
                                             start=(dt == 0),
                                             stop=(dt == NDT - 1))
                        sg = mp.tile([128, 512], BF, tag="sg", name="sg")
                        nc.scalar.activation(sg[:], ph1[:], AF.Sigmoid,
                                             bias=bs1f[:, ft:ft + 1])
                        t1h = mp.tile([128, 512], BF, tag="t1h", name="t1h")
                        nc.vector.tensor_scalar(t1h[:], ph1[:], 1.0,
                                                bs1f[:, ft:ft + 1],
                                                ALU.mult, ALU.add)
                        t3h = mp.tile([128, 512], BF, tag="t3h", name="t3h")
                        nc.vector.tensor_scalar(t3h[:], ph3[:], 1.0,
                                                bs3f[:, ft:ft + 1],
                                                ALU.mult, ALU.add)
                        a_t = mp.tile([128, 512], BF, tag="a_t", name="a_t")
                        nc.vector.tensor_tensor(out=a_t[:], in0=t1h[:],
                                                in1=sg[:], op=ALU.mult)
                        nc.vector.tensor_tensor(out=hs[:, ft, sl], in0=a_t[:],
                                                in1=t3h[:], op=ALU.mult)
                ws2_sb = mp.tile([128, NFT_S, D], BF, tag="ws2_sb")
                nc.sync.dma_start(out=ws2_sb[:],
                                  in_=t["ws2"][:].rearrange("f p d -> p f d"))
                for tt in range(NTT):
                    ys = wsp.tile([128, D], BF, tag="ys", name="ys")
                    for dc in range(4):
                        sl = slice(dc * 512, (dc + 1) * 512)
                        pys = psm.tile([128, 512], FP, tag="ps_ys", name="pys")
                        for ft in range(NFT_S):
                            nc.tensor.matmul(
                                pys[:], hs[:, ft, tt * 128:(tt + 1) * 128],
                                ws2_sb[:, ft, sl],
                                start=(ft == 0), stop=False)
                        b_s2 = wsp.tile([1, 512], BF, tag="b2t", name="b_s2")
                        nc.sync.dma_start(out=b_s2[:], in_=t["bs2_8"][0:1, sl])
                        nc.tensor.matmul(pys[:], ones_row[:, :128],
                                         b_s2[:], start=False, stop=True)
                        nc.scalar.activation(ys[:, sl], pys[:], AF.Copy)
                    tsl = slice(tt * 128, (tt + 1) * 128)
                    nc.sync.dma_start(out=d_yp0[tsl, :], in_=ys[:, :HD])
                    nc.sync.dma_start(out=d_yp1[tsl, :], in_=ys[:, HD:])

                # ---- routed experts: h for both, then ye per D-half ----
                b1fs = {"A": load(mp, "b1A"), "B": load(mp, "b1B")}
                b3fs = {"A": load(mp, "b3A"), "B": load(mp, "b3B")}
                hTs = {}
                for sfx, cap in (("A", CAP_A), ("B", CAP_B)):
                    XeT = XeTs[sfx]
                    XeTv = XeT[:].rearrange("p a b -> p (a b)").rearrange(
                        "p (c n i) -> p c i n", c=8, i=2)
                    hT = mp.tile([128, NFT + 1, cap], F8, tag=f"hT{sfx}",
                                 name=f"hT{sfx}")
                    hTs[sfx] = hT
                    nc.vector.memset(hT[:, NFT, :], 0)
                    chunks = [(0, 512), (512, cap)] if cap > 512 else [(0, cap)]
                    for ft in range(NFT):
                        w1_t = wsp.tile([128, 8, 2, 128], F8, tag="w1t",
                                        name="w1_t")
                        nc.sync.dma_start(out=w1_t[:], in_=t[f"w1{sfx}"][ft])
                        w3_t = wsp.tile([128, 8, 2, 128], F8, tag="w3t",
                                        name="w3_t")
                        nc.sync.dma_start(out=w3_t[:], in_=t[f"w3{sfx}"][ft])
                        for (lo, hi) in chunks:
                            w = hi - lo
                            ph1 = psm.tile([128, 512], FP, tag="ps_h1",
                                           name="ph1")
                            ph3 = psm.tile([128, 512], FP, tag="ps_h3",
                                           name="ph3")
                            for c8 in range(8):
                                nc.tensor.matmul(
                                    ph1[:, :w], w1_t[:, c8, :, :],
                                    XeTv[:, c8, :, lo:hi],
                                    start=(c8 == 0), stop=(c8 == 7),
                                    perf_mode=PM.DoubleRow)
                                nc.tensor.matmul(
                                    ph3[:, :w], w3_t[:, c8, :, :],
                                    XeTv[:, c8, :, lo:hi],
                                    start=(c8 == 0), stop=(c8 == 7),
                                    perf_mode=PM.DoubleRow)
                            b1f, b3f = b1fs[sfx], b3fs[sfx]
                            sg = mp.tile([128, 512], BF, tag="sg", name="sg")
                            nc.scalar.activation(sg[:, :w], ph1[:, :w],
                                                 AF.Sigmoid, scale=1.0 / WS,
                                                 bias=b1f[:, ft:ft + 1])
                            t1h = mp.tile([128, 512], BF, tag="t1h", name="t1h")
                            nc.vector.tensor_scalar(t1h[:, :w], ph1[:, :w],
                                                    1.0 / WS,
                                                    b1f[:, ft:ft + 1],
                                                    ALU.mult, ALU.add)
                            t3h = mp.tile([128, 512], BF, tag="t3h", name="t3h")
                            nc.vector.tensor_scalar(t3h[:, :w], ph3[:, :w],
                                                    1.0 / WS,
                                                    b3f[:, ft:ft + 1],
                                                    ALU.mult, ALU.add)
                            a_t = mp.tile([128, 512], BF, tag="a_t", name="a_t")
                            nc.vector.tensor_tensor(out=a_t[:, :w],
                                                    in0=t1h[:, :w],
                                                    in1=sg[:, :w], op=ALU.mult)
                            nc.vector.tensor_tensor(
                                out=hT[:, ft, lo:hi], in0=a_t[:, :w],
                                in1=t3h[:, :w], op=ALU.mult)

                cmy = tc.tile_pool(name="psy", bufs=1, space="PSUM")
                psy = cmy.__enter__()
                for half, d_yph, dcs in ((0, d_yp0, (0, 1)), (1, d_yp1, (2, 3))):
                    for sfx, cap, nit, base, it_base in (
                            ("A", CAP_A, NIT_A, 0, 0),
                            ("B", CAP_B, NIT_B, CAP_A, NIT_A)):
                        hT = hTs[sfx]
                        ye = mp.tile([128, nit, HD], BF, tag="ye",
                                     name=f"ye{sfx}{half}")
                        for dci, dc in enumerate(dcs):
                            sl = slice(dc * 512, (dc + 1) * 512)
                            osl = slice(dci * 512, (dci + 1) * 512)
                            pyes = [psy.tile([128, 512], FP, tag=f"ps_ye{i}",
                                             name=f"ps_ye_{sfx}{dc}_{i}")
                                    for i in range(nit)]
                            for ftp in range(NFTP):
                                w2_t = wsp.tile([128, 2, 512], F8, tag="w2t",
                                                name="w2_t")
                                nc.sync.dma_start(out=w2_t[:],
                                                  in_=t[f"w2{sfx}"][ftp, :, :, sl])
                                for it in range(nit):
                                    nc.tensor.matmul(
                                        pyes[it][:],
                                        hT[:, 2 * ftp:2 * ftp + 2,
                                           it * 128:(it + 1) * 128],
                                        w2_t[:], start=(ftp == 0), stop=False,
                                        perf_mode=PM.DoubleRow)
                            b_2 = wsp.tile([1, 512], BF, tag="b2t", name="b_2")
                            nc.sync.dma_start(out=b_2[:],
                                              in_=t[f"b2{sfx}"][0:1, sl])
                            for it in range(nit):
                                nc.tensor.matmul(pyes[it][:], ones_row[:, :128],
                                                 b_2[:], start=False, stop=True)
                                nc.scalar.activation(
                                    ye[:, it, osl], pyes[it][:], AF.Copy,
                                    scale=gws_its[it_base + it][:])
                        nc.gpsimd.dma_scatter_add(
                            out_ap=d_yph[:], in_ap=ye[:],
                            idxs_ap=idx_sb[:, base // 16:(base + cap) // 16],
                            num_idxs=cap, num_idxs_reg=cap, elem_size=HD,
                            queue_num=(0 if sfx == "A" else 1))
                    d_rsh = d_rs0 if half == 0 else d_rs1
                    nc.gpsimd.collective_compute(
                        "ReduceScatter", ALU.add, replica_groups=rg,
                        ins=[(d_yp0 if half == 0 else d_yp1)[:]],
                        outs=[d_rsh[:]])
                cmy.__exit__(None, None, None)

                # ---- final: residual + output ----
                rs_sb = mp.tile([128, D], BF, tag="rs_sb")
                nc.sync.dma_start(out=rs_sb[:, :HD], in_=d_rs0[:])
                nc.sync.dma_start(out=rs_sb[:, HD:], in_=d_rs1[:])
                nc.vector.tensor_tensor(out=x2_mine[:], in0=rs_sb[:],
                                        in1=x2_mine[:], op=ALU.add)
                nc.sync.dma_start(out=out_ext[:], in_=x2_mine[:])

    nc.compile()
    return nc


# --------------------------------------------------------------------------
# host-side input prep
# --------------------------------------------------------------------------

def _tile_w1(w):
    nft = w.shape[1] // 128
    return np.ascontiguousarray(
        w.reshape(NDT, 128, nft, 128).transpose(2, 1, 0, 3))


def _tile_w1_fp8(w):
    # [D, F] -> [NFT, 128(p), 8(c), 2(i), 128(f)] with d = c*256 + 2p + i
    return np.ascontiguousarray(
        (w * WS).astype(fp8).reshape(8, 128, 2, NFT, 128)
        .transpose(3, 1, 0, 2, 4))


def _tile_w2_fp8(w):
    # [F, D] -> [NFTP, 128(p), 2(i), D] with f = (2*ftp + i)*128 + p, pad to 12
    wp = np.zeros((NFTP * 2 * 128, D), np.float32)
    wp[:F] = w
    return np.ascontiguousarray(
        (wp * WS).astype(fp8).reshape(NFTP, 2, 128, D).transpose(0, 2, 1, 3))


def _prep_in_maps(inputs):
    f32 = lambda a: np.ascontiguousarray(np.asarray(a, dtype=np.float32))
    tobf = lambda a: np.ascontiguousarray(np.asarray(a, dtype=np.float32)
                                          .astype(bf16))
    x = f32(inputs["x"]).reshape(T, D)
    mask = f32(inputs["mask"])
    wq_w, wq_b = f32(inputs["wq_w"]), f32(inputs["wq_b"])
    wk_w, wk_b = f32(inputs["wk_w"]), f32(inputs["wk_b"])
    wv_w, wv_b = f32(inputs["wv_w"]), f32(inputs["wv_b"])
    wo_w, wo_b = f32(inputs["wo_w"]), f32(inputs["wo_b"])
    attn_g, ffn_g = f32(inputs["attn_g"]), f32(inputs["ffn_g"])
    gate_w, gate_b = f32(inputs["gate_w"]), f32(inputs["gate_b"])
    e_w1, e_b1 = f32(inputs["e_w1"]), f32(inputs["e_b1"])
    e_w2, e_b2 = f32(inputs["e_w2"]), f32(inputs["e_b2"])
    e_w3, e_b3 = f32(inputs["e_w3"]), f32(inputs["e_b3"])
    s_w1, s_b1 = f32(inputs["s_w1"]), f32(inputs["s_b1"])
    s_w2, s_b2 = f32(inputs["s_w2"]), f32(inputs["s_b2"])
    s_w3, s_b3 = f32(inputs["s_w3"]), f32(inputs["s_b3"])

    s_w1p = np.zeros((D, FS_PAD), np.float32); s_w1p[:, :FS] = s_w1
    s_w3p = np.zeros((D, FS_PAD), np.float32); s_w3p[:, :FS] = s_w3
    s_b1p = np.zeros(FS_PAD, np.float32); s_b1p[:FS] = s_b1
    s_b3p = np.zeros(FS_PAD, np.float32); s_b3p[:FS] = s_b3
    s_w2p = np.zeros((FS_PAD, D), np.float32); s_w2p[:FS] = s_w2

    i_idx = np.arange(T)[:, None]
    j_idx = np.arange(T)[None, :]
    rel = np.where(i_idx >= j_idx, -(i_idx - j_idx).astype(np.float32), 0.0)
    ident = np.eye(128, dtype=np.float32)
    tri_incl = (np.arange(128)[:, None] <= np.arange(128)[None, :]) \
        .astype(np.float32)
    tri_s8 = (np.arange(8)[:, None] < np.arange(8)[None, :]).astype(np.float32)
    iota_bc = np.tile(np.arange(CAP_A, dtype=np.float32), (128, 1))
    iota_t = (np.arange(NTT)[None, :] * 128
              + np.arange(128)[:, None]).astype(np.float32)

    in_maps = []
    for c in range(NCORES):
        eA, eB = A_EXPERTS[c], B_EXPERTS[c]
        slope = 2.0 ** (-(c + 1))
        selA = np.zeros(E, np.float32); selA[eA] = 1.0
        selB = np.zeros(E, np.float32); selB[eB] = 1.0
        ssel = np.zeros(NTT, np.float32); ssel[c] = 1.0
        fs_lo = c * FS_SLICE
        fs_hi = fs_lo + FS_SLICE
        m = {
            "x_full": x,
            "am": (mask + slope * rel).astype(np.float32),
            "wq": tobf(wq_w[:, c * DK:(c + 1) * DK]),
            "wk": tobf(wk_w[:, c * DK:(c + 1) * DK]),
            "wv": tobf(wv_w[:, c * DV:(c + 1) * DV]),
            "bq": tobf(wq_b[c * DK:(c + 1) * DK]).reshape(1, DK),
            "bk": tobf(wk_b[c * DK:(c + 1) * DK]).reshape(1, DK),
            "bv": tobf(wv_b[c * DV:(c + 1) * DV]).reshape(1, DV),
            "wo": tobf(wo_w),
            "wo_b": tobf(wo_b).reshape(1, D),
            "g1": np.tile(attn_g, (128, 1)),
            "g2": np.tile(ffn_g, (128, 1)),
            "gate_wT": np.ascontiguousarray(gate_w.T),
            "gate_b3": np.tile(gate_b, (128, NTT, 1)).astype(np.float32),
            "selA3": np.tile(selA, (128, NTT, 1)).astype(np.float32),
            "selB3": np.tile(selB, (128, NTT, 1)).astype(np.float32),
            "slice_sel": np.tile(ssel, (128, 1)),
            "ws1": _tile_w1(tobf(s_w1p[:, fs_lo:fs_hi])),
            "ws3": _tile_w1(tobf(s_w3p[:, fs_lo:fs_hi])),
            "ws2": tobf(s_w2p[fs_lo:fs_hi]).reshape(NFT_S, 128, D),
            "bs1": np.ascontiguousarray(
                s_b1p[fs_lo:fs_hi].reshape(NFT_S, 128).T),
            "bs3": np.ascontiguousarray(
                s_b3p[fs_lo:fs_hi].reshape(NFT_S, 128).T),
            "bs2_8": tobf(s_b2 / 8.0).reshape(1, D),
            "ident": ident.astype(bf16),
            "ident_f": ident,
            "tri_incl": tri_incl,
            "tri_s8": tri_s8,
            "ones8": np.ones((8, 128), np.float32),
            "ones_col": np.ones((128, 1), np.float32),
            "iota_bc": iota_bc,
            "iota_t": iota_t,
            "ones_row": np.ones((1, 1024), bf16),
        }
        for sfx, e in (("A", eA), ("B", eB)):
            m[f"w1{sfx}"] = _tile_w1_fp8(e_w1[e])
            m[f"w3{sfx}"] = _tile_w1_fp8(e_w3[e])
            m[f"w2{sfx}"] = _tile_w2_fp8(e_w2[e])
            m[f"b1{sfx}"] = np.ascontiguousarray(
                e_b1[e].reshape(NFT, 128).T.astype(np.float32))
            m[f"b3{sfx}"] = np.ascontiguousarray(
                e_b3[e].reshape(NFT, 128).T.astype(np.float32))
            m[f"b2{sfx}"] = tobf(e_b2[e] * WS).reshape(1, D)
        in_maps.append(m)
    return in_maps


def _get_nc():
    if "nc" not in _CACHE:
        _CACHE["nc"] = _build_nc()
    return _CACHE["nc"]


def kernel(trace=False, **inputs):
    nc = _get_nc()
    in_maps = _prep_in_maps(inputs)
    res = run_bass_kernel_spmd(nc, in_maps, core_ids=list(range(NCORES)),
                               trace=trace)
    out = np.concatenate([res.results[c]["out"] for c in range(NCORES)],
                         axis=0).reshape(1, T, D).astype(np.float32)
    if trace:
        return out, res
    return out
